# revision 1
# baseline (speedup 1.0000x reference)
"""AirportGNN (4-layer GAT, N=50000, E=800000) on 8 TRN2 NeuronCores.

Sharding: nodes 6250/core; edges assigned to the dst-owner core, dst-sorted,
128-edge chunks aligned to 128-node blocks, split into 2 streams by src<N/2
(dma_gather int16 index limit). All cores run ONE graph: per-(block,stream)
chunk counts are padded to the cross-core max so the structure is uniform.

Per layer:
  phase A: [hp|a_s|a_d] = h @ [W | W@Asrc | W@Adst] per local node -> 512B-row
           node table -> AllGather.
  phase B: dma_gather src rows per chunk (4 SWDGE queues); alpha = a_s[src] +
           a_d[dst] + a_e(edge) (a_d expanded via host-built one-hot Mt matmul,
           a_e precomputed once on device); leaky-relu; exp (no max-subtraction,
           args are O(1)); segment softmax-sum via matmul against on-the-fly
           indicator M accumulated in per-block PSUM; [num|den(|ea for layer0)]
           evacuated to per-stream staging.
  epilogue (batched over blocks): self-loop term added densely, normalize,
           +bias, ELU, +residual, LayerNorm.
Readout: final table AllGather; zone rows gathered per half + reordered via
one-hot perm matmuls; 3 MLP heads in feature-major layout; core 0's output.
"""
import os
import sys
import numpy as np

if '/opt/trn_rl_repo' not in sys.path:
    sys.path.insert(0, '/opt/trn_rl_repo')

import ml_dtypes
import concourse.bacc as bacc
import concourse.mybir as mybir
import concourse.tile as tile
from concourse.ap import AP
from concourse.bass_utils import run_bass_kernel_spmd
from concourse._compat import cdiv

F32 = mybir.dt.float32
BF16 = mybir.dt.bfloat16
I16 = mybir.dt.int16
AF = mybir.ActivationFunctionType
OP = mybir.AluOpType

NCORES = 8
USE_LRELU = os.environ.get('KERNEL_LRELU', '') == '1'
HID, H, C = 96, 4, 24
NEG = 0.2
GRP = 16
LAYERS = 4


def _bmid(ap, n, axis=1):
    """Insert a 0-step broadcast dim of size n at position axis of an AP."""
    l = [list(x) for x in ap.ap]
    return AP(ap.tensor, ap.offset, l[:axis] + [[0, n]] + l[axis:])


def _btail(ap, n):
    """Replace a trailing size-1 dim with a 0-step broadcast dim of size n."""
    l = [list(x) for x in ap.ap]
    assert l[-1][1] == 1, l
    return AP(ap.tensor, ap.offset, l[:-1] + [[0, n]])


def _wrap_idx(ix, n):
    a = np.zeros((16, cdiv(n, 16)), np.int16)
    ix = np.asarray(ix, np.int64)
    for p in range(16):
        v = ix[p::16]
        a[p, :len(v)] = v.astype(np.int16)
    return np.tile(a, (8, 1))


def host_prep(inputs, N, E):
    NL = N // NCORES
    HALF = N // 2
    NBLK = cdiv(NL, 128)
    LASTN = NL - (NBLK - 1) * 128

    x = np.asarray(inputs['x'], np.float32)
    ei = np.asarray(inputs['edge_index'], np.int64)
    ea_np = np.asarray(inputs['edge_attr'], np.float32)
    zone = np.asarray(inputs['zone_idx'], np.int64)
    src_all, dst_all = ei[0], ei[1]
    f32 = lambda k: np.asarray(inputs[k], np.float32)

    # ---- per-core raw edge lists, grouped by (core, block, stream) ----
    raw = []  # raw[k][s][b] = (src, dst_rel, ea)
    for k in range(NCORES):
        lo = k * NL
        sel = (dst_all >= lo) & (dst_all < lo + NL)
        es, ed = src_all[sel], dst_all[sel] - lo
        o = np.argsort(ed, kind='stable')
        es, ed, eat = es[o], ed[o], ea_np[sel][o]
        per = [[None] * NBLK for _ in range(2)]
        for s in (0, 1):
            for b in range(NBLK):
                m = (ed // 128 == b) & ((es < HALF) == (s == 0))
                bs, bd, be = es[m], ed[m] - b * 128, eat[m]
                o2 = np.argsort(bs, kind='stable')
                per[s][b] = (bs[o2], bd[o2], be[o2])
        raw.append(per)

    NWIN = NBLK
    ncb = [[max(cdiv(max(len(raw[k][s][b][0]), 1), 128) for k in range(NCORES))
            for b in range(NBLK)] for s in range(2)]
    nch = []
    for s in (0, 1):
        tot = sum(ncb[s])
        pad = (-tot) % GRP
        ncb[s][NBLK - 1] += pad
        nch.append(tot + pad)
    ncht = nch[0] + nch[1]
    chunk_win = []
    win_ranges = [[], []]
    for s in (0, 1):
        c = 0
        for b in range(NBLK):
            win_ranges[s].append((c, c + ncb[s][b]))
            chunk_win += [b] * ncb[s][b]
            c += ncb[s][b]

    # ---- shared weights ----
    W1a = np.concatenate([f32('in_w1'), f32('in_b1')[None, :]], 0)
    W2a = np.concatenate([f32('in_w2'), f32('in_b2')[None, :]], 0)
    Wcat = np.zeros((HID, LAYERS * 104), np.float32)
    w_eh = np.zeros((4, LAYERS, H), np.float32)
    bias_b = np.zeros((128, LAYERS * 96), np.float32)
    lns_b = np.zeros((128, LAYERS * 96), np.float32)
    lnb_b = np.zeros((128, LAYERS * 96), np.float32)
    for l in range(LAYERS):
        W = f32('conv_w')[l]
        As = np.zeros((HID, H), np.float32)
        Ad = np.zeros((HID, H), np.float32)
        for hh in range(H):
            As[hh * C:(hh + 1) * C, hh] = f32('conv_att_src')[l, hh]
            Ad[hh * C:(hh + 1) * C, hh] = f32('conv_att_dst')[l, hh]
        Wcat[:, l * 104:l * 104 + 96] = W
        Wcat[:, l * 104 + 96:l * 104 + 100] = W @ As
        Wcat[:, l * 104 + 100:l * 104 + 104] = W @ Ad
        w_eh[:, l, :] = np.einsum('ahc,hc->ah',
                                  f32('conv_lin_edge')[l].reshape(4, H, C),
                                  f32('conv_att_edge')[l])
        bias_b[:, l * 96:(l + 1) * 96] = f32('conv_bias')[l][None, :]
        lns_b[:, l * 96:(l + 1) * 96] = f32('norm_scale')[l][None, :]
        lnb_b[:, l * 96:(l + 1) * 96] = f32('norm_bias')[l][None, :]
    weh_b = np.tile(w_eh.reshape(1, 4 * LAYERS * H), (128, 1)).astype(np.float32)

    def head_aug(pre):
        return (np.concatenate([f32(pre + '_w1'), f32(pre + '_b1')[None]], 0),
                np.concatenate([f32(pre + '_w2'), f32(pre + '_b2')[None]], 0),
                np.concatenate([f32(pre + '_w3'), f32(pre + '_b3')[None]], 0))
    heads = [head_aug('cong'), head_aug('delay'), head_aug('jit')]

    iota_row = np.tile(np.arange(128, dtype=np.float32), (128, 1))
    ident = np.eye(128, dtype=np.float32)

    # ---- zone prep (shared) ----
    nz = len(zone)
    NZC = cdiv(nz, 128)
    zlists = [[j for j in range(nz) if (zone[j] < HALF) == (h == 0)] for h in range(2)]
    zidx = []
    for h in range(2):
        ix = [int(zone[j]) - h * HALF for j in zlists[h]]
        ix += [0] * (NZC * 128 - len(ix))
        zidx.append(_wrap_idx(ix, NZC * 128))
    perms = {}
    for h in range(2):
        for r, j in enumerate(zlists[h]):
            key = (h, r // 128, j // 128)
            if key not in perms:
                perms[key] = np.zeros((128, 128), np.float32)
            perms[key][r % 128, j % 128] = 1.0
    perm_keys = sorted(perms.keys())
    perm_mats = (np.concatenate([perms[k] for k in perm_keys], 0)
                 if perm_keys else np.zeros((128, 128), np.float32))

    # ---- per-core arrays ----
    in_maps = []
    for k in range(NCORES):
        idx_arr = np.zeros((128, ncht // 8 * 64), np.int16)
        Mt = np.zeros((128, ncht, 128), np.float32)
        ea_chunk = np.zeros((128, ncht, 4), np.float32)
        flat_idx = np.zeros((ncht, 128), np.int64)
        for s in (0, 1):
            off = 0 if s == 0 else nch[0]
            for w in range(NWIN):
                bsrc, bdst, bea = raw[k][s][w]
                c0 = win_ranges[s][w][0] + off
                for i in range(0, len(bsrc), 128):
                    ci = c0 + i // 128
                    n = min(128, len(bsrc) - i)
                    flat_idx[ci, :n] = bsrc[i:i + n] - s * HALF
                    Mt[bdst[i:i + n], ci, np.arange(n)] = 1.0
                    ea_chunk[:n, ci, :] = bea[i:i + n]
        for g in range(ncht // 8):
            ix = flat_idx[g * 8:(g + 1) * 8].reshape(-1)
            idx_arr[:, g * 64:(g + 1) * 64] = _wrap_idx(ix, 8 * 128)

        deg = np.bincount(dst_all[(dst_all >= k * NL) & (dst_all < (k + 1) * NL)] - k * NL,
                          minlength=NL).astype(np.float32)
        inv_deg = (1.0 / np.clip(deg, 1.0, None)).astype(np.float32)
        inv_deg_b = np.pad(inv_deg, (0, NBLK * 128 - NL)).reshape(NBLK, 128).T.copy()

        xa = x[k * NL:(k + 1) * NL]
        xT_aug = np.ascontiguousarray(
            np.concatenate([xa.T, np.ones((1, NL), np.float32)], 0))

        M_arr = np.ascontiguousarray(Mt.transpose(2, 1, 0))  # [edge, chunk, node64]
        im = {
            'xT_aug': xT_aug, 'idx': idx_arr,
            'Mt': np.ascontiguousarray(Mt.reshape(128, ncht * 128).astype(ml_dtypes.bfloat16)),
            'Ms': np.ascontiguousarray(M_arr.reshape(128, ncht * 128).astype(ml_dtypes.bfloat16)),
            'ea_chunk': np.ascontiguousarray(ea_chunk.reshape(128, ncht * 4)),
            'ea_bf': np.ascontiguousarray(ea_chunk.reshape(128, ncht * 4).astype(ml_dtypes.bfloat16)),
            'inv_deg': np.ascontiguousarray(inv_deg_b),
            'W1a': W1a, 'W2a': W2a, 'Wcat': Wcat, 'weh': weh_b,
            'bias_b': bias_b, 'lns_b': lns_b, 'lnb_b': lnb_b,
            'iota': iota_row, 'ident': ident,
            'zidx0': zidx[0], 'zidx1': zidx[1],
            'perms': perm_mats.astype(ml_dtypes.bfloat16),
        }
        for hi_, (w1, w2, w3) in enumerate(heads):
            im[f'hw1_{hi_}'], im[f'hw2_{hi_}'], im[f'hw3_{hi_}'] = w1, w2, w3
        in_maps.append(im)

    meta = {
        'N': N, 'NL': NL, 'HALF': HALF, 'NBLK': NBLK, 'LASTN': LASTN, 'NZC': NZC,
        'NWIN': NWIN,
        'nch': nch, 'ncht': ncht, 'chunk_win': chunk_win, 'win_ranges': win_ranges,
        'perm_keys': perm_keys, 'nperm': max(len(perm_keys), 1),
        'head_dims': [2, 1, 1],
    }
    return in_maps, meta


def build(meta):
    N, NL, HALF = meta['N'], meta['NL'], meta['HALF']
    NBLK, LASTN, NZC = meta['NBLK'], meta['LASTN'], meta['NZC']
    nch, ncht = meta['nch'], meta['ncht']
    chunk_win, win_ranges = meta['chunk_win'], meta['win_ranges']
    NWIN = meta['NWIN']
    perm_keys, nperm = meta['perm_keys'], meta['nperm']
    head_dims = meta['head_dims']

    nc = bacc.Bacc('TRN2', target_bir_lowering=False, debug=False, num_swdge_queues=4)
    P = lambda n, s, d, o=False: nc.declare_dram_parameter(n, s, d, isOutput=o)

    xT_aug = P('xT_aug', [13, NL], F32)
    idx_e = P('idx', [128, ncht // 8 * 64], I16)
    Mt_e = P('Mt', [128, ncht * 128], BF16)
    Ms_e = P('Ms', [128, ncht * 128], BF16)
    ea_e = P('ea_chunk', [128, ncht * 4], F32)
    eabf_e = P('ea_bf', [128, ncht * 4], BF16)
    inv_deg_e = P('inv_deg', [128, NBLK], F32)
    W1a_e = P('W1a', [13, 96], F32)
    W2a_e = P('W2a', [97, 96], F32)
    Wcat_e = P('Wcat', [HID, LAYERS * 104], F32)
    weh_e = P('weh', [128, 64], F32)
    bias_e = P('bias_b', [128, LAYERS * 96], F32)
    lns_e = P('lns_b', [128, LAYERS * 96], F32)
    lnb_e = P('lnb_b', [128, LAYERS * 96], F32)
    iota_e = P('iota', [128, 128], F32)
    ident_e = P('ident', [128, 128], F32)
    zidx_e = [P('zidx0', [128, NZC * 8], I16), P('zidx1', [128, NZC * 8], I16)]
    perms_e = P('perms', [nperm * 128, 128], BF16)
    hw = [(P(f'hw1_{i}', [97, 96], F32), P(f'hw2_{i}', [97, 48], F32),
           P(f'hw3_{i}', [49, head_dims[i]], F32)) for i in range(3)]
    out_e = P('out', [64, 6, 4], F32, o=True)

    table_loc = nc.dram_tensor('table_loc', [NWIN * 128, 128], BF16)
    table = nc.dram_tensor('table', [N, 128], BF16, addr_space='Shared')
    ae_dram = nc.dram_tensor('ae_dram', [128, ncht * 16], F32)
    rg = [list(range(NCORES))]
    ae_view = ae_dram.ap().rearrange('p (c s) -> p c s', s=16)

    with tile.TileContext(nc) as tc:
        with tc.tile_pool(name='const', bufs=1) as cpool, \
             tc.tile_pool(name='big', bufs=1) as bpool, \
             tc.tile_pool(name='st', bufs=1) as spool, \
             tc.tile_pool(name='ps', bufs=1, space='PSUM') as pp:

            def ctile(name, src_ap, shape):
                t = cpool.tile(shape, F32, name=name, tag=name)
                nc.sync.dma_start(t[:], src_ap)
                return t

            ident_t = ctile('ident_t', ident_e[:], [128, 128])
            weh_t = ctile('weh_t', weh_e[:], [128, 64])
            inv_deg_t = ctile('inv_deg_t', inv_deg_e[:], [128, NBLK])
            Wcat_t = ctile('Wcat_t', Wcat_e[:].rearrange('p (l o) -> p l o', l=LAYERS),
                           [HID, LAYERS, 104])
            bias_t = ctile('bias_t', bias_e[:].rearrange('p (l o) -> p l o', l=LAYERS),
                           [128, LAYERS, 96])
            lns_t = ctile('lns_t', lns_e[:].rearrange('p (l o) -> p l o', l=LAYERS),
                          [128, LAYERS, 96])
            lnb_t = ctile('lnb_t', lnb_e[:].rearrange('p (l o) -> p l o', l=LAYERS),
                          [128, LAYERS, 96])

            h_cur = bpool.tile([128, NBLK, 96], F32, name='h0', tag='h', bufs=2)
            gq = [0]  # global gather counter: queue = i%4 must track Tile's DMASW sem i%8
            eps_t = cpool.tile([128, 1], F32, name='eps_t', tag='eps_t')
            nc.vector.memset(eps_t[:], 1e-5)
            loop_attr = bpool.tile([128, NBLK, 4], F32, name='loop_attr')
            loop_alpha = bpool.tile([128, NBLK, 16], F32, name='loop_alpha')

            # ---------- init-only work (big tiles share stg slots, dead by phase B) ----------
            zt = spool.tile([128, 128], BF16, name='zt', tag='zt')
            nc.vector.memset(zt[:], 0.0)
            for b in range(NWIN):
                nc.sync.dma_start(table_loc[b * 128:(b + 1) * 128, :], zt[:])

            # ---------- input MLP ----------
            t_T = bpool.tile([97, NL], F32, name='t_T', tag='stg0')
            W1a_t = ctile('W1a_t', W1a_e[:], [13, 96])
            W2a_t = ctile('W2a_t', W2a_e[:], [97, 96])
            xT_t = bpool.tile([13, NL], F32, name='xT_t', tag='stg1')
            nc.sync.dma_start(xT_t[:], xT_aug[:])
            for i in range(cdiv(NL, 512)):
                w = min(512, NL - i * 512)
                ps1 = pp.tile([96, 512], F32, name='ps1', tag='pT', bufs=1)
                nc.tensor.matmul(ps1[:, 0:w], W1a_t[:], xT_t[:, i * 512:i * 512 + w],
                                 start=True, stop=True)
                nc.scalar.activation(t_T[0:96, i * 512:i * 512 + w], ps1[:, 0:w], AF.Relu)
            nc.vector.memset(t_T[96:97, :], 1.0)
            for b in range(NBLK):
                nb = 128 if b < NBLK - 1 else LASTN
                ps2 = pp.tile([128, 96], F32, name='ps2', tag='pA', bufs=1)
                nc.tensor.matmul(ps2[0:nb, :], t_T[:, b * 128:b * 128 + nb], W2a_t[:],
                                 start=True, stop=True)
                if nb < 128:
                    nc.vector.memset(h_cur[96:128, b, :], 0.0)
                nc.vector.tensor_copy(h_cur[0:nb, b, :], ps2[0:nb, :])

            # ---------- a_e precompute ----------
            for g in range(cdiv(ncht, 32)):
                c0, c1 = g * 32, min((g + 1) * 32, ncht)
                w = c1 - c0
                eat = spool.tile([128, 32, 4], F32, name='eat', tag='eat', bufs=2)
                nc.sync.dma_start(eat[:, 0:w, :],
                                  ea_e[:].rearrange('p (c a) -> p c a', a=4)[:, c0:c1, :])
                aet = spool.tile([128, 32, 16], F32, name='aet', tag='aet', bufs=2)
                tmp = spool.tile([128, 32, 16], F32, name='aetmp', tag='aetmp', bufs=2)
                for a in range(4):
                    dst = aet if a == 0 else tmp
                    nc.vector.tensor_tensor(
                        out=dst[:, 0:w, :],
                        in0=_btail(eat[:, 0:w, a:a + 1], 16),
                        in1=_bmid(weh_t[:, a * 16:(a + 1) * 16], w), op=OP.mult)
                    if a > 0:
                        nc.vector.tensor_tensor(out=aet[:, 0:w, :], in0=aet[:, 0:w, :],
                                                in1=tmp[:, 0:w, :], op=OP.add)
                nc.sync.dma_start(ae_view[:, c0:c1, :].rearrange('p c s -> p (c s)'),
                                  aet[:, 0:w, :])

            # ---------- layers ----------
            for l in range(LAYERS):
                wid = 104 if l == 0 else 100

                # ---- phase A ----
                phA = bpool.tile([128, NBLK, 104], F32, name=f'phA{l}', tag='phA')
                for b in range(NBLK):
                    nb = 128 if b < NBLK - 1 else LASTN
                    pt = pp.tile([96, 128], F32, name='pt', tag='pT', bufs=1)
                    nc.tensor.transpose(pt[:], h_cur[:, b, :], ident_t[:])
                    hT = spool.tile([96, 128], F32, name='hT', tag='hT', bufs=3)
                    nc.vector.tensor_copy(hT[:], pt[:])
                    pa = pp.tile([128, 104], F32, name='pa', tag='pA', bufs=1)
                    nc.tensor.matmul(pa[0:nb, :], hT[:, 0:nb], Wcat_t[:, l, :],
                                     start=True, stop=True)
                    if nb < 128:
                        nc.vector.memset(phA[96:128, b, :], 0.0)
                    nc.vector.tensor_copy(phA[0:nb, b, :], pa[0:nb, :])
                    tbf = spool.tile([128, 104], BF16, name='tbf', tag='tbf', bufs=3)
                    nc.scalar.activation(tbf[0:nb, :], pa[0:nb, :], AF.Copy)
                    nc.sync.dma_start(table_loc[b * 128:b * 128 + nb, 0:104],
                                      tbf[0:nb, :])
                adw = spool.tile([128, NWIN, 4], BF16, name='adw', tag='adw', bufs=2)
                nc.sync.dma_start(
                    adw[:], table_loc.ap().rearrange('(w q) f -> w q f', q=128)
                    [:, :, 100:104].rearrange('w q f -> q w f'))
                nc.gpsimd.collective_compute(
                    'AllGather', OP.bypass, replica_groups=rg,
                    ins=[table_loc[0:NL, :].opt()], outs=[table.ap().opt()])

                # ---- phase B ----
                stg = [bpool.tile([128, NBLK, 104], F32, name=f'stg{s}_{l}', tag=f'stg{s}')
                       for s in (0, 1)]
                nc.vector.memset(stg[0][:], 0.0)
                nc.vector.memset(stg[1][:], 0.0)
                for s in (0, 1):
                    goff = (0 if s == 0 else nch[0] // GRP)
                    coff = 0 if s == 0 else nch[0]
                    tbl_half = table[s * HALF:(s + 1) * HALF, :]
                    cur_ps, cur_b = None, -1
                    for g in range(nch[s] // GRP):
                        cg0 = coff + g * GRP
                        ixt = spool.tile([128, GRP * 8], I16, name='ixt', tag='ixt', bufs=4)
                        nc.sync.dma_start(ixt[:], idx_e[:, (goff + g) * GRP * 8:(goff + g + 1) * GRP * 8])
                        gt = spool.tile([128, GRP, 128], BF16, name='gt', tag='gt', bufs=3)
                        for half_g in range(GRP // 8):
                            nc.gpsimd.dma_gather(
                                gt[:, half_g * 8:(half_g + 1) * 8, :], tbl_half,
                                ixt[:, half_g * 64:(half_g + 1) * 64], 1024, 1024,
                                128, queue_num=gq[0] % 4)
                            gq[0] += 1
                        mtt = spool.tile([128, GRP, 128], BF16, name='mtt', tag='mtt', bufs=3)
                        nc.sync.dma_start(
                            mtt[:], Mt_e[:].rearrange('p (c e) -> p c e', e=128)
                            [:, cg0:cg0 + GRP, :])
                        aet2 = spool.tile([128, GRP, 4], F32, name='aet2', tag='aet2', bufs=4)
                        nc.sync.dma_start(aet2[:], ae_view[:, cg0:cg0 + GRP, l * 4:(l + 1) * 4])

                        adp = pp.tile([128, GRP * 4], F32, name='adp', tag='pD', bufs=4)
                        for c in range(GRP):
                            w = chunk_win[cg0 + c]
                            nc.tensor.matmul(adp[:, c * 4:(c + 1) * 4], mtt[:, c, :],
                                             adw[:, w, :], start=True, stop=True)
                        alpha = spool.tile([128, GRP, 4], F32, name='alpha', tag='alpha', bufs=4)
                        nc.vector.tensor_tensor(
                            out=alpha[:], in0=gt[:, :, 96:100],
                            in1=adp[:].rearrange('p (c f) -> p c f', c=GRP), op=OP.add)
                        nc.vector.tensor_tensor(out=alpha[:], in0=alpha[:], in1=aet2[:],
                                                op=OP.add)
                        msg = spool.tile([128, GRP, 104], BF16, name='msg', tag='msg', bufs=3)
                        e1 = spool.tile([128, GRP, 4], F32, name='e1', tag='e1', bufs=4)
                        nc.scalar.activation(e1[:], alpha[:], AF.Exp)
                        e2 = spool.tile([128, GRP, 4], F32, name='e2', tag='e2', bufs=4)
                        nc.scalar.activation(e2[:], alpha[:], AF.Exp, scale=NEG)
                        ex = spool.tile([128, GRP, 4], F32, name='ex', tag='ex', bufs=4)
                        nc.vector.tensor_tensor(out=ex[:], in0=e1[:], in1=e2[:], op=OP.max)
                        nc.vector.tensor_tensor(out=msg[:, :, 96:100], in0=e1[:], in1=e2[:],
                                                op=OP.max)
                        for c in range(GRP):
                            nc.vector.tensor_tensor(
                                out=msg[:, c, 0:96].rearrange('p (h r) -> p h r', h=4),
                                in0=gt[:, c, 0:96].rearrange('p (h r) -> p h r', h=4),
                                in1=ex[:, c, :].broadcast_to([128, 4, 24]), op=OP.mult)
                        if l == 0:
                            nc.sync.dma_start(
                                msg[:, :, 100:104],
                                eabf_e[:].rearrange('p (c a) -> p c a', a=4)
                                [:, cg0:cg0 + GRP, :])
                        Mb = spool.tile([128, GRP, 128], BF16, name='Mb', tag='Mb', bufs=3)
                        nc.sync.dma_start(
                            Mb[:], Ms_e[:].rearrange('p (c e) -> p c e', e=128)
                            [:, cg0:cg0 + GRP, :])
                        for c in range(GRP):
                            cb = chunk_win[cg0 + c]
                            if cb != cur_b:
                                assert cur_ps is None
                                cur_ps = pp.tile([128, 104], F32, name='psb', tag='pB', bufs=2)
                                cur_b = cb
                            first = (cg0 + c) == coff + win_ranges[s][cb][0]
                            last = (cg0 + c) == coff + win_ranges[s][cb][1] - 1
                            nc.tensor.matmul(cur_ps[:, 0:wid], Mb[:, c, :], msg[:, c, 0:wid],
                                             start=first, stop=last)
                            if last:
                                nc.vector.tensor_copy(stg[s][:, cb, 0:wid], cur_ps[:, 0:wid])
                                cur_ps, cur_b = None, -1
                    assert cur_ps is None

                # ---- epilogue (batched over blocks) ----
                nc.vector.tensor_tensor(out=stg[0][:], in0=stg[0][:], in1=stg[1][:], op=OP.add)
                sA = stg[0]
                if l == 0:
                    nc.vector.tensor_tensor(
                        out=loop_attr[:], in0=sA[:, :, 100:104],
                        in1=inv_deg_t[:].broadcast_to([128, NBLK, 4]), op=OP.mult)
                # loop_alpha[p,b,l*4+h] = sum_a loop_attr[p,b,a] * weh[a, l*4+h]
                la_tmp = spool.tile([128, NBLK, 16], F32, name='la_tmp', tag='la_tmp')
                for a in range(4):
                    dst = loop_alpha if a == 0 else la_tmp
                    nc.vector.tensor_tensor(
                        out=dst[:],
                        in0=_btail(loop_attr[:, :, a:a + 1], 16),
                        in1=_bmid(weh_t[:, a * 16:(a + 1) * 16], NBLK), op=OP.mult)
                    if a > 0:
                        nc.vector.tensor_tensor(out=loop_alpha[:], in0=loop_alpha[:],
                                                in1=la_tmp[:], op=OP.add)
                all_ = spool.tile([128, NBLK, 4], F32, name='all_', tag='all_')
                nc.vector.tensor_tensor(out=all_[:], in0=loop_alpha[:, :, l * 4:(l + 1) * 4],
                                        in1=phA[:, :, 96:100], op=OP.add)
                nc.vector.tensor_tensor(out=all_[:], in0=all_[:], in1=phA[:, :, 100:104],
                                        op=OP.add)
                lk2 = spool.tile([128, NBLK, 4], F32, name='lk2', tag='lk2')
                nc.vector.tensor_scalar_mul(lk2[:], all_[:], NEG)
                nc.vector.tensor_tensor(out=all_[:], in0=all_[:], in1=lk2[:], op=OP.max)
                exl = spool.tile([128, NBLK, 4], F32, name='exl', tag='exl')
                nc.scalar.activation(exl[:], all_[:], AF.Exp)
                # num += exl * hp ; den += exl
                t96 = bpool.tile([128, NBLK, 96], F32, name='t96', tag='stg1')
                nc.vector.tensor_tensor(
                    out=t96[:].rearrange('p b (h r) -> p b h r', h=4),
                    in0=phA[:, :, 0:96].rearrange('p b (h r) -> p b h r', h=4),
                    in1=exl[:].broadcast_to([128, NBLK, 4, 24]), op=OP.mult)
                nc.vector.tensor_tensor(out=sA[:, :, 0:96], in0=sA[:, :, 0:96], in1=t96[:],
                                        op=OP.add)
                nc.vector.tensor_tensor(out=sA[:, :, 96:100], in0=sA[:, :, 96:100],
                                        in1=exl[:], op=OP.add)
                rec = spool.tile([128, NBLK, 4], F32, name='rec', tag='rec')
                nc.vector.reciprocal(rec[:], sA[:, :, 96:100])
                # out = num*rec + bias
                nc.vector.tensor_tensor(
                    out=t96[:].rearrange('p b (h r) -> p b h r', h=4),
                    in0=sA[:, :, 0:96].rearrange('p b (h r) -> p b h r', h=4),
                    in1=rec[:].broadcast_to([128, NBLK, 4, 24]), op=OP.mult)
                nc.vector.tensor_tensor(out=t96[:], in0=t96[:],
                                        in1=_bmid(bias_t[:, l, :], NBLK), op=OP.add)
                # elu: (exp(min(x,0)) - 1) + max(x,0), then + res
                emn = bpool.tile([128, NBLK, 96], F32, name='emn', tag='phA')
                nc.vector.tensor_scalar_min(emn[:], t96[:], 0.0)
                nc.scalar.activation(emn[:], emn[:], AF.Exp)
                nc.vector.tensor_scalar_add(emn[:], emn[:], -1.0)
                nc.vector.tensor_scalar_max(t96[:], t96[:], 0.0)
                nc.vector.tensor_tensor(out=t96[:], in0=t96[:], in1=emn[:], op=OP.add)
                nc.vector.tensor_tensor(out=t96[:], in0=t96[:], in1=h_cur[:], op=OP.add)
                # LN
                mean = spool.tile([128, NBLK], F32, name='mean', tag='mean')
                nc.vector.tensor_reduce(mean[:], t96[:], axis=mybir.AxisListType.X, op=OP.add)
                nc.vector.tensor_scalar_mul(mean[:], mean[:], 1.0 / 96)
                nc.vector.tensor_tensor(out=t96[:], in0=t96[:],
                                        in1=mean[:].broadcast_to([128, NBLK, 96]),
                                        op=OP.subtract)
                sq = bpool.tile([128, NBLK, 96], F32, name='sq', tag='phA')
                nc.vector.tensor_tensor(out=sq[:], in0=t96[:], in1=t96[:], op=OP.mult)
                var = spool.tile([128, NBLK], F32, name='var', tag='var')
                nc.vector.tensor_reduce(var[:], sq[:], axis=mybir.AxisListType.X, op=OP.add)
                sd = spool.tile([128, NBLK], F32, name='sd', tag='sd')
                nc.scalar.activation(sd[:], var[:], AF.Sqrt, bias=eps_t[:, 0:1], scale=1.0 / 96)
                rstd = spool.tile([128, NBLK], F32, name='rstd', tag='rstd')
                nc.vector.reciprocal(rstd[:], sd[:])
                h_new = bpool.tile([128, NBLK, 96], F32, name=f'h{l + 1}', tag='h', bufs=2)
                nc.vector.tensor_tensor(out=t96[:], in0=t96[:],
                                        in1=rstd[:].broadcast_to([128, NBLK, 96]), op=OP.mult)
                nc.vector.tensor_tensor(out=t96[:], in0=t96[:],
                                        in1=_bmid(lns_t[:, l, :], NBLK), op=OP.mult)
                nc.vector.tensor_tensor(out=h_new[:], in0=t96[:],
                                        in1=_bmid(lnb_t[:, l, :], NBLK), op=OP.add)
                h_cur = h_new

            # ---------- readout ----------
            for b in range(NBLK):
                nb = 128 if b < NBLK - 1 else LASTN
                tbf2 = spool.tile([128, 96], BF16, name='tbf2', tag='tbf', bufs=3)
                nc.scalar.activation(tbf2[0:nb, :], h_cur[0:nb, b, :], AF.Copy)
                nc.sync.dma_start(table_loc[b * 128:b * 128 + nb, 0:96], tbf2[0:nb, :])
            nc.gpsimd.collective_compute(
                'AllGather', OP.bypass, replica_groups=rg,
                ins=[table_loc[0:NL, :].opt()], outs=[table.ap().opt()])

            zg = []
            for hh in range(2):
                zi = spool.tile([128, NZC * 8], I16, name=f'zi{hh}', tag=f'zi{hh}')
                nc.sync.dma_start(zi[:], zidx_e[hh][:])
                zgt = spool.tile([128, NZC, 128], BF16, name=f'zg{hh}', tag=f'zg{hh}')
                nc.gpsimd.dma_gather(zgt[:], table[hh * HALF:(hh + 1) * HALF, :], zi[:],
                                     NZC * 128, NZC * 128, 128, queue_num=gq[0] % 4)
                gq[0] += 1
                zg.append(zgt)

            z_T = spool.tile([97, NZC * 128], F32, name='z_T', tag='z_T')
            nc.vector.memset(z_T[96:97, :], 1.0)
            for jc in range(NZC):
                pz = pp.tile([128, 96], F32, name='pz', tag='pA', bufs=1)
                keys = [k for k in perm_keys if k[2] == jc]
                for i, (hh, ic, _) in enumerate(keys):
                    pi = perm_keys.index((hh, ic, jc))
                    pm = spool.tile([128, 128], BF16, name='pm', tag='pm', bufs=2)
                    nc.sync.dma_start(pm[:], perms_e[pi * 128:(pi + 1) * 128, :])
                    nc.tensor.matmul(pz[:], pm[:], zg[hh][:, ic, 0:96],
                                     start=(i == 0), stop=(i == len(keys) - 1))
                zs = spool.tile([128, 96], F32, name='zs', tag='zs', bufs=2)
                nc.vector.tensor_copy(zs[:], pz[:])
                ptz = pp.tile([96, 128], F32, name='ptz', tag='pT', bufs=1)
                nc.tensor.transpose(ptz[:], zs[:], ident_t[:])
                nc.vector.tensor_copy(z_T[0:96, jc * 128:(jc + 1) * 128], ptz[:])

            outS = spool.tile([128, NZC, 4], F32, name='outS', tag='outS')
            ooff = 0
            for hi_ in range(3):
                o = head_dims[hi_]
                w1t = spool.tile([97, 96], F32, name='w1t', tag='w1t', bufs=2)
                nc.sync.dma_start(w1t[:], hw[hi_][0][:])
                w2t = spool.tile([97, 48], F32, name='w2t', tag='w2t', bufs=2)
                nc.sync.dma_start(w2t[:], hw[hi_][1][:])
                w3t = spool.tile([48, o], F32, name='w3t', tag='w3t', bufs=2)
                nc.sync.dma_start(w3t[:], hw[hi_][2][0:48, :])
                b3t = spool.tile([4, 1], F32, name='b3t', tag='b3t', bufs=2)
                nc.sync.dma_start(b3t[0:o, :], hw[hi_][2][48:49, 0:o].rearrange('a b -> b a'))
                p1 = pp.tile([96, NZC * 128], F32, name='p1', tag='pT', bufs=1)
                nc.tensor.matmul(p1[:], w1t[:], z_T[:], start=True, stop=True)
                t1 = spool.tile([97, NZC * 128], F32, name='t1', tag='t1', bufs=2)
                nc.scalar.activation(t1[0:96, :], p1[:], AF.Relu)
                nc.vector.memset(t1[96:97, :], 1.0)
                p2 = pp.tile([48, NZC * 128], F32, name='p2', tag='pA', bufs=1)
                nc.tensor.matmul(p2[:], w2t[:], t1[:], start=True, stop=True)
                t2 = spool.tile([48, NZC * 128], F32, name='t2', tag='t2', bufs=2)
                nc.scalar.activation(t2[:], p2[:], AF.Relu)
                p3 = pp.tile([4, NZC * 128], F32, name='p3', tag='pD', bufs=4)
                nc.tensor.matmul(p3[0:o, :], w3t[:], t2[:], start=True, stop=True)
                oh = spool.tile([4, NZC * 128], F32, name='oh', tag='oh', bufs=2)
                nc.vector.tensor_scalar(out=oh[0:o, :], in0=p3[0:o, :],
                                        scalar1=b3t[0:o, 0:1], scalar2=None, op0=OP.add)
                for jc in range(NZC):
                    po = pp.tile([128, 4], F32, name='po', tag='pB', bufs=2)
                    nc.tensor.transpose(po[:, 0:o], oh[0:o, jc * 128:(jc + 1) * 128],
                                        ident_t[0:o, 0:o])
                    nc.vector.tensor_copy(outS[:, jc, ooff:ooff + o], po[:, 0:o])
                ooff += o
            nc.sync.dma_start(
                out_e.ap().rearrange('a z f -> (a z) f')
                    .rearrange('(c p) f -> p c f', p=128), outS[:])

    nc.compile()
    return nc


def _run(inputs, trace=False):
    N = int(np.asarray(inputs['x']).shape[0])
    E = int(np.asarray(inputs['edge_index']).shape[1])
    in_maps, meta = host_prep(inputs, N, E)
    nc = build(meta)
    res = run_bass_kernel_spmd(nc, in_maps, core_ids=list(range(NCORES)), trace=trace)
    return np.asarray(res.results[0]['out'], np.float32).reshape(64, 6, 4), res


def kernel(**inputs):
    return _run(inputs, trace=False)[0]



# revision 3
# speedup vs baseline: 1.0836x; 1.0836x over previous
"""AirportGNN (4-layer GAT, N=50000, E=800000) on 8 TRN2 NeuronCores.

Sharding: nodes 6250/core; edges assigned to the dst-owner core, dst-sorted,
128-edge chunks aligned to 128-node blocks, split into 2 streams by src<N/2
(dma_gather int16 index limit). All cores run ONE graph: per-(block,stream)
chunk counts are padded to the cross-core max so the structure is uniform.

Per layer:
  phase A: [hp|a_s|a_d] = h @ [W | W@Asrc | W@Adst] per local node -> 512B-row
           node table -> AllGather; a_d block rows kept in SBUF (adw_sb).
  phase B: dma_gather src rows per chunk (4 SWDGE queues). One-hot dst
           indicators are GENERATED ON CHIP (not loaded): Ms[e,d] via
           is_equal(dstcol, iota) on vector; Mt[d,e] via a K=1 PE broadcast
           of the dst row into PSUM + is_equal against the partition iota.
           alpha = a_s[src] + a_d[dst] (Mt matmul) + a_e (bf16, layer-major
           contiguous); leaky-relu via exp/max; segment softmax-sum via
           matmul against Ms accumulated in per-block PSUM; evacuated by the
           scalar engine to per-stream staging.
  epilogue (batched over blocks): self-loop term added densely, normalize,
           +bias, ELU, +residual, LayerNorm.
Readout: final table AllGather; zone rows gathered per half + reordered via
one-hot perm matmuls; 3 MLP heads in feature-major layout; core 0's output.
"""
import os
import sys
import numpy as np

if '/opt/trn_rl_repo' not in sys.path:
    sys.path.insert(0, '/opt/trn_rl_repo')

import ml_dtypes
import concourse.bacc as bacc
import concourse.mybir as mybir
import concourse.tile as tile
from concourse.ap import AP
from concourse.bass_utils import run_bass_kernel_spmd
from concourse._compat import cdiv

F32 = mybir.dt.float32
BF16 = mybir.dt.bfloat16
I16 = mybir.dt.int16
AF = mybir.ActivationFunctionType
OP = mybir.AluOpType

NCORES = 8
HID, H, C = 96, 4, 24
NEG = 0.2
GRP = 16
LAYERS = 4


def _bmid(ap, n, axis=1):
    """Insert a 0-step broadcast dim of size n at position axis of an AP."""
    l = [list(x) for x in ap.ap]
    return AP(ap.tensor, ap.offset, l[:axis] + [[0, n]] + l[axis:])


def _btail(ap, n):
    """Replace a trailing size-1 dim with a 0-step broadcast dim of size n."""
    l = [list(x) for x in ap.ap]
    assert l[-1][1] == 1, l
    return AP(ap.tensor, ap.offset, l[:-1] + [[0, n]])


def _wrap_idx(ix, n):
    a = np.zeros((16, cdiv(n, 16)), np.int16)
    ix = np.asarray(ix, np.int64)
    for p in range(16):
        v = ix[p::16]
        a[p, :len(v)] = v.astype(np.int16)
    return np.tile(a, (8, 1))


def host_prep(inputs, N, E):
    NL = N // NCORES
    HALF = N // 2
    NBLK = cdiv(NL, 128)
    LASTN = NL - (NBLK - 1) * 128

    x = np.asarray(inputs['x'], np.float32)
    ei = np.asarray(inputs['edge_index'], np.int64)
    ea_np = np.asarray(inputs['edge_attr'], np.float32)
    zone = np.asarray(inputs['zone_idx'], np.int64)
    src_all, dst_all = ei[0], ei[1]
    f32 = lambda k: np.asarray(inputs[k], np.float32)

    # ---- per-core raw edge lists, grouped by (core, block, stream) ----
    raw = []  # raw[k][s][b] = (src, dst_rel, ea)
    for k in range(NCORES):
        lo = k * NL
        sel = (dst_all >= lo) & (dst_all < lo + NL)
        es, ed = src_all[sel], dst_all[sel] - lo
        o = np.argsort(ed, kind='stable')
        es, ed, eat = es[o], ed[o], ea_np[sel][o]
        per = [[None] * NBLK for _ in range(2)]
        for s in (0, 1):
            for b in range(NBLK):
                m = (ed // 128 == b) & ((es < HALF) == (s == 0))
                bs, bd, be = es[m], ed[m] - b * 128, eat[m]
                o2 = np.argsort(bs, kind='stable')
                per[s][b] = (bs[o2], bd[o2], be[o2])
        raw.append(per)

    NWIN = NBLK
    ncb = [[max(cdiv(max(len(raw[k][s][b][0]), 1), 128) for k in range(NCORES))
            for b in range(NBLK)] for s in range(2)]
    nch = []
    for s in (0, 1):
        tot = sum(ncb[s])
        pad = (-tot) % GRP
        ncb[s][NBLK - 1] += pad
        nch.append(tot + pad)
    ncht = nch[0] + nch[1]
    chunk_win = []
    win_ranges = [[], []]
    for s in (0, 1):
        c = 0
        for b in range(NBLK):
            win_ranges[s].append((c, c + ncb[s][b]))
            chunk_win += [b] * ncb[s][b]
            c += ncb[s][b]

    # ---- shared weights ----
    W1a = np.concatenate([f32('in_w1'), f32('in_b1')[None, :]], 0)
    W2a = np.concatenate([f32('in_w2'), f32('in_b2')[None, :]], 0)
    Wcat = np.zeros((HID, LAYERS * 104), np.float32)
    w_eh = np.zeros((4, LAYERS, H), np.float32)
    bias_b = np.zeros((128, LAYERS * 96), np.float32)
    lns_b = np.zeros((128, LAYERS * 96), np.float32)
    lnb_b = np.zeros((128, LAYERS * 96), np.float32)
    for l in range(LAYERS):
        W = f32('conv_w')[l]
        As = np.zeros((HID, H), np.float32)
        Ad = np.zeros((HID, H), np.float32)
        for hh in range(H):
            As[hh * C:(hh + 1) * C, hh] = f32('conv_att_src')[l, hh]
            Ad[hh * C:(hh + 1) * C, hh] = f32('conv_att_dst')[l, hh]
        Wcat[:, l * 104:l * 104 + 96] = W
        Wcat[:, l * 104 + 96:l * 104 + 100] = W @ As
        Wcat[:, l * 104 + 100:l * 104 + 104] = W @ Ad
        w_eh[:, l, :] = np.einsum('ahc,hc->ah',
                                  f32('conv_lin_edge')[l].reshape(4, H, C),
                                  f32('conv_att_edge')[l])
        bias_b[:, l * 96:(l + 1) * 96] = f32('conv_bias')[l][None, :]
        lns_b[:, l * 96:(l + 1) * 96] = f32('norm_scale')[l][None, :]
        lnb_b[:, l * 96:(l + 1) * 96] = f32('norm_bias')[l][None, :]
    weh_b = np.tile(w_eh.reshape(1, 4 * LAYERS * H), (128, 1)).astype(np.float32)

    def head_aug(pre):
        return (np.concatenate([f32(pre + '_w1'), f32(pre + '_b1')[None]], 0),
                np.concatenate([f32(pre + '_w2'), f32(pre + '_b2')[None]], 0),
                np.concatenate([f32(pre + '_w3'), f32(pre + '_b3')[None]], 0))
    heads = [head_aug('cong'), head_aug('delay'), head_aug('jit')]

    ident = np.eye(128, dtype=np.float32)
    iota_bf = np.tile(np.arange(128, dtype=ml_dtypes.bfloat16), (128, 1))
    iotap_bf = np.arange(128, dtype=ml_dtypes.bfloat16).reshape(128, 1)
    ones1 = np.ones((1, 128), ml_dtypes.bfloat16)

    # ---- zone prep (shared) ----
    nz = len(zone)
    NZC = cdiv(nz, 128)
    zlists = [[j for j in range(nz) if (zone[j] < HALF) == (h == 0)] for h in range(2)]
    zidx = []
    for h in range(2):
        ix = [int(zone[j]) - h * HALF for j in zlists[h]]
        ix += [0] * (NZC * 128 - len(ix))
        zidx.append(_wrap_idx(ix, NZC * 128))
    perms = {}
    for h in range(2):
        for r, j in enumerate(zlists[h]):
            key = (h, r // 128, j // 128)
            if key not in perms:
                perms[key] = np.zeros((128, 128), np.float32)
            perms[key][r % 128, j % 128] = 1.0
    perm_keys = sorted(perms.keys())
    perm_mats = (np.concatenate([perms[k] for k in perm_keys], 0)
                 if perm_keys else np.zeros((128, 128), np.float32))

    # ---- per-core arrays ----
    in_maps = []
    for k in range(NCORES):
        idx_arr = np.zeros((128, ncht // 8 * 64), np.int16)
        ea_chunk = np.zeros((128, ncht, 4), np.float32)
        flat_idx = np.zeros((ncht, 128), np.int64)
        dstc = np.full((128, ncht), -1.0, ml_dtypes.bfloat16)
        dstrow = np.full((1, ncht * 128), -1.0, ml_dtypes.bfloat16)
        for s in (0, 1):
            off = 0 if s == 0 else nch[0]
            for w in range(NWIN):
                bsrc, bdst, bea = raw[k][s][w]
                c0 = win_ranges[s][w][0] + off
                for i in range(0, len(bsrc), 128):
                    ci = c0 + i // 128
                    n = min(128, len(bsrc) - i)
                    flat_idx[ci, :n] = bsrc[i:i + n] - s * HALF
                    dstc[:n, ci] = bdst[i:i + n].astype(ml_dtypes.bfloat16)
                    dstrow[0, ci * 128:ci * 128 + n] = \
                        bdst[i:i + n].astype(ml_dtypes.bfloat16)
                    ea_chunk[:n, ci, :] = bea[i:i + n]
        for g in range(ncht // 8):
            ix = flat_idx[g * 8:(g + 1) * 8].reshape(-1)
            idx_arr[:, g * 64:(g + 1) * 64] = _wrap_idx(ix, 8 * 128)

        deg = np.bincount(dst_all[(dst_all >= k * NL) & (dst_all < (k + 1) * NL)] - k * NL,
                          minlength=NL).astype(np.float32)
        inv_deg = (1.0 / np.clip(deg, 1.0, None)).astype(np.float32)
        inv_deg_b = np.pad(inv_deg, (0, NBLK * 128 - NL)).reshape(NBLK, 128).T.copy()

        xa = x[k * NL:(k + 1) * NL]
        xT_aug = np.ascontiguousarray(
            np.concatenate([xa.T, np.ones((1, NL), np.float32)], 0))

        im = {
            'xT_aug': xT_aug, 'idx': idx_arr,
            'dstc': np.ascontiguousarray(dstc),
            'dstrow': np.ascontiguousarray(dstrow),
            'ea_chunk': np.ascontiguousarray(ea_chunk.reshape(128, ncht * 4)),
            'ea_bf': np.ascontiguousarray(
                ea_chunk.reshape(128, ncht * 4).astype(ml_dtypes.bfloat16)),
            'inv_deg': np.ascontiguousarray(inv_deg_b),
            'W1a': W1a, 'W2a': W2a, 'Wcat': Wcat, 'weh': weh_b,
            'bias_b': bias_b, 'lns_b': lns_b, 'lnb_b': lnb_b,
            'ident': ident, 'iota_bf': iota_bf, 'iotap_bf': iotap_bf,
            'ones1': ones1,
            'zidx0': zidx[0], 'zidx1': zidx[1],
            'perms': perm_mats.astype(ml_dtypes.bfloat16),
        }
        for hi_, (w1, w2, w3) in enumerate(heads):
            im[f'hw1_{hi_}'], im[f'hw2_{hi_}'], im[f'hw3_{hi_}'] = w1, w2, w3
        in_maps.append(im)

    meta = {
        'N': N, 'NL': NL, 'HALF': HALF, 'NBLK': NBLK, 'LASTN': LASTN, 'NZC': NZC,
        'NWIN': NWIN,
        'nch': nch, 'ncht': ncht, 'chunk_win': chunk_win, 'win_ranges': win_ranges,
        'perm_keys': perm_keys, 'nperm': max(len(perm_keys), 1),
        'head_dims': [2, 1, 1],
    }
    return in_maps, meta


def build(meta):
    N, NL, HALF = meta['N'], meta['NL'], meta['HALF']
    NBLK, LASTN, NZC = meta['NBLK'], meta['LASTN'], meta['NZC']
    nch, ncht = meta['nch'], meta['ncht']
    chunk_win, win_ranges = meta['chunk_win'], meta['win_ranges']
    NWIN = meta['NWIN']
    perm_keys, nperm = meta['perm_keys'], meta['nperm']
    head_dims = meta['head_dims']

    nc = bacc.Bacc('TRN2', target_bir_lowering=False, debug=False, num_swdge_queues=4)
    P = lambda n, s, d, o=False: nc.declare_dram_parameter(n, s, d, isOutput=o)

    xT_aug = P('xT_aug', [13, NL], F32)
    idx_e = P('idx', [128, ncht // 8 * 64], I16)
    dstc_e = P('dstc', [128, ncht], BF16)
    dstrow_e = P('dstrow', [1, ncht * 128], BF16)
    ea_e = P('ea_chunk', [128, ncht * 4], F32)
    eabf_e = P('ea_bf', [128, ncht * 4], BF16)
    inv_deg_e = P('inv_deg', [128, NBLK], F32)
    W1a_e = P('W1a', [13, 96], F32)
    W2a_e = P('W2a', [97, 96], F32)
    Wcat_e = P('Wcat', [HID, LAYERS * 104], F32)
    weh_e = P('weh', [128, 64], F32)
    bias_e = P('bias_b', [128, LAYERS * 96], F32)
    lns_e = P('lns_b', [128, LAYERS * 96], F32)
    lnb_e = P('lnb_b', [128, LAYERS * 96], F32)
    ident_e = P('ident', [128, 128], F32)
    iota_e = P('iota_bf', [128, 128], BF16)
    iotap_e = P('iotap_bf', [128, 1], BF16)
    ones1_e = P('ones1', [1, 128], BF16)
    zidx_e = [P('zidx0', [128, NZC * 8], I16), P('zidx1', [128, NZC * 8], I16)]
    perms_e = P('perms', [nperm * 128, 128], BF16)
    hw = [(P(f'hw1_{i}', [97, 96], F32), P(f'hw2_{i}', [97, 48], F32),
           P(f'hw3_{i}', [49, head_dims[i]], F32)) for i in range(3)]
    out_e = P('out', [64, 6, 4], F32, o=True)

    table_loc = nc.dram_tensor('table_loc', [NWIN * 128, 128], BF16)
    table = nc.dram_tensor('table', [N, 128], BF16, addr_space='Shared')
    # a_e logits, bf16, layer-major: [128, L, ncht, 4]
    ae_dram = nc.dram_tensor('ae_dram', [128, LAYERS * ncht * 4], BF16)
    rg = [list(range(NCORES))]
    ae_view = ae_dram.ap().rearrange('p (l c a) -> p l c a', l=LAYERS, a=4)

    with tile.TileContext(nc) as tc:
        with tc.tile_pool(name='const', bufs=1) as cpool, \
             tc.tile_pool(name='big', bufs=1) as bpool, \
             tc.tile_pool(name='st', bufs=1) as spool, \
             tc.tile_pool(name='ps', bufs=1, space='PSUM') as pp:

            def ctile(name, src_ap, shape, dt=F32):
                t = cpool.tile(shape, dt, name=name, tag=name)
                nc.sync.dma_start(t[:], src_ap)
                return t

            ident_t = ctile('ident_t', ident_e[:], [128, 128])
            weh_t = ctile('weh_t', weh_e[:], [128, 64])
            inv_deg_t = ctile('inv_deg_t', inv_deg_e[:], [128, NBLK])
            iota_t = ctile('iota_t', iota_e[:], [128, 128], BF16)
            iotap_t = ctile('iotap_t', iotap_e[:], [128, 1], BF16)
            ones1_t = ctile('ones1_t', ones1_e[:], [1, 128], BF16)
            Wcat_t = ctile('Wcat_t', Wcat_e[:].rearrange('p (l o) -> p l o', l=LAYERS),
                           [HID, LAYERS, 104])
            bias_t = ctile('bias_t', bias_e[:].rearrange('p (l o) -> p l o', l=LAYERS),
                           [128, LAYERS, 96])
            lns_t = ctile('lns_t', lns_e[:].rearrange('p (l o) -> p l o', l=LAYERS),
                          [128, LAYERS, 96])
            lnb_t = ctile('lnb_t', lnb_e[:].rearrange('p (l o) -> p l o', l=LAYERS),
                          [128, LAYERS, 96])

            h_cur = bpool.tile([128, NBLK, 96], F32, name='h0', tag='h', bufs=2)
            gq = [0]  # global gather counter: queue = i%4 must track Tile's DMASW sem i%8
            eps_t = cpool.tile([128, 1], F32, name='eps_t', tag='eps_t')
            nc.vector.memset(eps_t[:], 1e-5)
            loop_attr = bpool.tile([128, NBLK, 4], F32, name='loop_attr')
            loop_alpha = bpool.tile([128, NBLK, 16], F32, name='loop_alpha')

            # ---------- init-only work (big tiles share stg slots, dead by phase B) ----------
            zt = spool.tile([128, 128], BF16, name='zt', tag='zt')
            nc.vector.memset(zt[:], 0.0)
            for b in range(NWIN):
                nc.sync.dma_start(table_loc[b * 128:(b + 1) * 128, :], zt[:])

            # ---------- input MLP ----------
            t_T = bpool.tile([97, NL], F32, name='t_T', tag='stg0')
            W1a_t = ctile('W1a_t', W1a_e[:], [13, 96])
            W2a_t = ctile('W2a_t', W2a_e[:], [97, 96])
            xT_t = bpool.tile([13, NL], F32, name='xT_t', tag='stg1')
            nc.sync.dma_start(xT_t[:], xT_aug[:])
            for i in range(cdiv(NL, 512)):
                w = min(512, NL - i * 512)
                ps1 = pp.tile([96, 512], F32, name='ps1', tag='pT', bufs=1)
                nc.tensor.matmul(ps1[:, 0:w], W1a_t[:], xT_t[:, i * 512:i * 512 + w],
                                 start=True, stop=True)
                nc.scalar.activation(t_T[0:96, i * 512:i * 512 + w], ps1[:, 0:w], AF.Relu)
            nc.vector.memset(t_T[96:97, :], 1.0)
            for b in range(NBLK):
                nb = 128 if b < NBLK - 1 else LASTN
                ps2 = pp.tile([128, 96], F32, name='ps2', tag='pA', bufs=1)
                nc.tensor.matmul(ps2[0:nb, :], t_T[:, b * 128:b * 128 + nb], W2a_t[:],
                                 start=True, stop=True)
                if nb < 128:
                    nc.vector.memset(h_cur[96:128, b, :], 0.0)
                nc.vector.tensor_copy(h_cur[0:nb, b, :], ps2[0:nb, :])

            # ---------- a_e precompute (bf16, layer-major) ----------
            for g in range(cdiv(ncht, 32)):
                c0, c1 = g * 32, min((g + 1) * 32, ncht)
                w = c1 - c0
                eat = spool.tile([128, 32, 4], F32, name='eat', tag='eat', bufs=2)
                nc.sync.dma_start(eat[:, 0:w, :],
                                  ea_e[:].rearrange('p (c a) -> p c a', a=4)[:, c0:c1, :])
                aet = spool.tile([128, 32, 16], F32, name='aet', tag='aet', bufs=2)
                tmp = spool.tile([128, 32, 16], F32, name='aetmp', tag='aetmp', bufs=2)
                for a in range(4):
                    dst = aet if a == 0 else tmp
                    nc.vector.tensor_tensor(
                        out=dst[:, 0:w, :],
                        in0=_btail(eat[:, 0:w, a:a + 1], 16),
                        in1=_bmid(weh_t[:, a * 16:(a + 1) * 16], w), op=OP.mult)
                    if a > 0:
                        nc.vector.tensor_tensor(out=aet[:, 0:w, :], in0=aet[:, 0:w, :],
                                                in1=tmp[:, 0:w, :], op=OP.add)
                for l in range(LAYERS):
                    abf = spool.tile([128, 32, 4], BF16, name='abf', tag='abf', bufs=3)
                    nc.scalar.activation(abf[:, 0:w, :], aet[:, 0:w, l * 4:(l + 1) * 4],
                                         AF.Copy)
                    nc.sync.dma_start(
                        ae_view[:, l, c0:c1, :].rearrange('p c a -> p (c a)'),
                        abf[:, 0:w, :])

            # ---------- layers ----------
            for l in range(LAYERS):
                wid = 104 if l == 0 else 100

                # ---- phase A ----
                phA = bpool.tile([128, NBLK, 104], F32, name=f'phA{l}', tag='phA')
                adw_sb = bpool.tile([128, NWIN, 4], BF16, name=f'adw{l}', tag='adw_sb',
                                    bufs=2)
                nc.vector.memset(adw_sb[:], 0.0)
                for b in range(NBLK):
                    nb = 128 if b < NBLK - 1 else LASTN
                    pt = pp.tile([96, 128], F32, name='pt', tag='pT', bufs=1)
                    nc.tensor.transpose(pt[:], h_cur[:, b, :], ident_t[:])
                    hT = spool.tile([96, 128], F32, name='hT', tag='hT', bufs=3)
                    nc.vector.tensor_copy(hT[:], pt[:])
                    pa = pp.tile([128, 104], F32, name='pa', tag='pA', bufs=1)
                    nc.tensor.matmul(pa[0:nb, :], hT[:, 0:nb], Wcat_t[:, l, :],
                                     start=True, stop=True)
                    if nb < 128:
                        nc.vector.memset(phA[96:128, b, :], 0.0)
                    nc.vector.tensor_copy(phA[0:nb, b, :], pa[0:nb, :])
                    nc.scalar.activation(adw_sb[0:nb, b, :], pa[0:nb, 100:104], AF.Copy)
                    tbf = spool.tile([128, 104], BF16, name='tbf', tag='tbf', bufs=3)
                    nc.scalar.activation(tbf[0:nb, :], pa[0:nb, :], AF.Copy)
                    nc.sync.dma_start(table_loc[b * 128:b * 128 + nb, 0:104],
                                      tbf[0:nb, :])
                nc.gpsimd.collective_compute(
                    'AllGather', OP.bypass, replica_groups=rg,
                    ins=[table_loc[0:NL, :].opt()], outs=[table.ap().opt()])

                # ---- phase B ----
                stg = [bpool.tile([128, NBLK, 104], F32, name=f'stg{s}_{l}', tag=f'stg{s}')
                       for s in (0, 1)]
                nc.vector.memset(stg[0][:], 0.0)
                nc.vector.memset(stg[1][:], 0.0)
                for s in (0, 1):
                    goff = (0 if s == 0 else nch[0] // GRP)
                    coff = 0 if s == 0 else nch[0]
                    tbl_half = table[s * HALF:(s + 1) * HALF, :]
                    cur_ps, cur_b = None, -1
                    for g in range(nch[s] // GRP):
                        cg0 = coff + g * GRP
                        ixt = spool.tile([128, GRP * 8], I16, name='ixt', tag='ixt', bufs=4)
                        nc.sync.dma_start(ixt[:], idx_e[:, (goff + g) * GRP * 8:(goff + g + 1) * GRP * 8])
                        dstc_t = spool.tile([128, GRP], BF16, name='dstc_t', tag='dstc_t',
                                            bufs=4)
                        nc.sync.dma_start(dstc_t[:], dstc_e[:, cg0:cg0 + GRP])
                        drow = spool.tile([1, GRP * 128], BF16, name='drow', tag='drow',
                                          bufs=4)
                        nc.sync.dma_start(drow[:],
                                          dstrow_e[0:1, cg0 * 128:(cg0 + GRP) * 128])
                        aet2 = spool.tile([128, GRP, 4], BF16, name='aet2', tag='aet2',
                                          bufs=4)
                        nc.sync.dma_start(
                            aet2[:], ae_view[:, l, cg0:cg0 + GRP, :]
                            .rearrange('p c a -> p (c a)'))
                        gt = spool.tile([128, GRP, 128], BF16, name='gt', tag='gt', bufs=2)
                        for half_g in range(GRP // 8):
                            nc.gpsimd.dma_gather(
                                gt[:, half_g * 8:(half_g + 1) * 8, :], tbl_half,
                                ixt[:, half_g * 64:(half_g + 1) * 64], 1024, 1024,
                                128, queue_num=gq[0] % 4)
                            gq[0] += 1

                        # on-chip one-hot generation
                        Ms_sb = spool.tile([128, GRP, 128], BF16, name='Ms_sb',
                                           tag='Ms_sb', bufs=2)
                        nc.vector.tensor_tensor(
                            out=Ms_sb[:],
                            in0=_btail(dstc_t[:].rearrange('p (c o) -> p c o', o=1), 128),
                            in1=_bmid(iota_t[:], GRP), op=OP.is_equal)
                        Mt_sb = spool.tile([128, GRP, 128], BF16, name='Mt_sb',
                                           tag='Mt_sb', bufs=2)
                        for q in range(GRP // 4):
                            Dq = pp.tile([128, 512], F32, name='Dq', tag='pDq', bufs=2)
                            nc.tensor.matmul(Dq[:], ones1_t[0:1, :],
                                             drow[0:1, q * 512:(q + 1) * 512],
                                             start=True, stop=True)
                            nc.vector.tensor_tensor(
                                out=Mt_sb[:, q * 4:(q + 1) * 4, :],
                                in0=Dq[:].rearrange('p (c e) -> p c e', e=128),
                                in1=_bmid(_btail(iotap_t[:], 128), 4), op=OP.is_equal)

                        adp = pp.tile([128, GRP * 4], F32, name='adp', tag='pD', bufs=2)
                        for c in range(GRP):
                            w = chunk_win[cg0 + c]
                            nc.tensor.matmul(adp[:, c * 4:(c + 1) * 4], Mt_sb[:, c, :],
                                             adw_sb[:, w, :], start=True, stop=True)
                        alpha = spool.tile([128, GRP, 4], F32, name='alpha', tag='alpha', bufs=4)
                        nc.vector.tensor_tensor(
                            out=alpha[:], in0=gt[:, :, 96:100],
                            in1=adp[:].rearrange('p (c f) -> p c f', c=GRP), op=OP.add)
                        nc.vector.tensor_tensor(out=alpha[:], in0=alpha[:], in1=aet2[:],
                                                op=OP.add)
                        msg = spool.tile([128, GRP, 104], BF16, name='msg', tag='msg', bufs=2)
                        e1 = spool.tile([128, GRP, 4], F32, name='e1', tag='e1', bufs=4)
                        nc.scalar.activation(e1[:], alpha[:], AF.Exp)
                        e2 = spool.tile([128, GRP, 4], F32, name='e2', tag='e2', bufs=4)
                        nc.scalar.activation(e2[:], alpha[:], AF.Exp, scale=NEG)
                        ex = spool.tile([128, GRP, 4], F32, name='ex', tag='ex', bufs=4)
                        nc.vector.tensor_tensor(out=ex[:], in0=e1[:], in1=e2[:], op=OP.max)
                        nc.vector.tensor_tensor(out=msg[:, :, 96:100], in0=e1[:], in1=e2[:],
                                                op=OP.max)
                        for c in range(GRP):
                            nc.vector.tensor_tensor(
                                out=msg[:, c, 0:96].rearrange('p (h r) -> p h r', h=4),
                                in0=gt[:, c, 0:96].rearrange('p (h r) -> p h r', h=4),
                                in1=ex[:, c, :].broadcast_to([128, 4, 24]), op=OP.mult)
                        if l == 0:
                            nc.sync.dma_start(
                                msg[:, :, 100:104],
                                eabf_e[:].rearrange('p (c a) -> p c a', a=4)
                                [:, cg0:cg0 + GRP, :])
                        for c in range(GRP):
                            cb = chunk_win[cg0 + c]
                            if cb != cur_b:
                                assert cur_ps is None
                                cur_ps = pp.tile([128, 104], F32, name='psb', tag='pB', bufs=2)
                                cur_b = cb
                            first = (cg0 + c) == coff + win_ranges[s][cb][0]
                            last = (cg0 + c) == coff + win_ranges[s][cb][1] - 1
                            nc.tensor.matmul(cur_ps[:, 0:wid], Ms_sb[:, c, :], msg[:, c, 0:wid],
                                             start=first, stop=last)
                            if last:
                                nc.scalar.activation(stg[s][:, cb, 0:wid], cur_ps[:, 0:wid],
                                                     AF.Copy)
                                cur_ps, cur_b = None, -1
                    assert cur_ps is None

                # ---- epilogue (batched over blocks) ----
                nc.vector.tensor_tensor(out=stg[0][:], in0=stg[0][:], in1=stg[1][:], op=OP.add)
                sA = stg[0]
                if l == 0:
                    nc.vector.tensor_tensor(
                        out=loop_attr[:], in0=sA[:, :, 100:104],
                        in1=inv_deg_t[:].broadcast_to([128, NBLK, 4]), op=OP.mult)
                # loop_alpha[p,b,l*4+h] = sum_a loop_attr[p,b,a] * weh[a, l*4+h]
                la_tmp = spool.tile([128, NBLK, 16], F32, name='la_tmp', tag='la_tmp')
                for a in range(4):
                    dst = loop_alpha if a == 0 else la_tmp
                    nc.vector.tensor_tensor(
                        out=dst[:],
                        in0=_btail(loop_attr[:, :, a:a + 1], 16),
                        in1=_bmid(weh_t[:, a * 16:(a + 1) * 16], NBLK), op=OP.mult)
                    if a > 0:
                        nc.vector.tensor_tensor(out=loop_alpha[:], in0=loop_alpha[:],
                                                in1=la_tmp[:], op=OP.add)
                all_ = spool.tile([128, NBLK, 4], F32, name='all_', tag='all_')
                nc.vector.tensor_tensor(out=all_[:], in0=loop_alpha[:, :, l * 4:(l + 1) * 4],
                                        in1=phA[:, :, 96:100], op=OP.add)
                nc.vector.tensor_tensor(out=all_[:], in0=all_[:], in1=phA[:, :, 100:104],
                                        op=OP.add)
                lk2 = spool.tile([128, NBLK, 4], F32, name='lk2', tag='lk2')
                nc.vector.tensor_scalar_mul(lk2[:], all_[:], NEG)
                nc.vector.tensor_tensor(out=all_[:], in0=all_[:], in1=lk2[:], op=OP.max)
                exl = spool.tile([128, NBLK, 4], F32, name='exl', tag='exl')
                nc.scalar.activation(exl[:], all_[:], AF.Exp)
                # num += exl * hp ; den += exl
                t96 = bpool.tile([128, NBLK, 96], F32, name='t96', tag='stg1')
                nc.vector.tensor_tensor(
                    out=t96[:].rearrange('p b (h r) -> p b h r', h=4),
                    in0=phA[:, :, 0:96].rearrange('p b (h r) -> p b h r', h=4),
                    in1=exl[:].broadcast_to([128, NBLK, 4, 24]), op=OP.mult)
                nc.vector.tensor_tensor(out=sA[:, :, 0:96], in0=sA[:, :, 0:96], in1=t96[:],
                                        op=OP.add)
                nc.vector.tensor_tensor(out=sA[:, :, 96:100], in0=sA[:, :, 96:100],
                                        in1=exl[:], op=OP.add)
                rec = spool.tile([128, NBLK, 4], F32, name='rec', tag='rec')
                nc.vector.reciprocal(rec[:], sA[:, :, 96:100])
                # out = num*rec + bias
                nc.vector.tensor_tensor(
                    out=t96[:].rearrange('p b (h r) -> p b h r', h=4),
                    in0=sA[:, :, 0:96].rearrange('p b (h r) -> p b h r', h=4),
                    in1=rec[:].broadcast_to([128, NBLK, 4, 24]), op=OP.mult)
                nc.vector.tensor_tensor(out=t96[:], in0=t96[:],
                                        in1=_bmid(bias_t[:, l, :], NBLK), op=OP.add)
                # elu: (exp(min(x,0)) - 1) + max(x,0), then + res
                emn = bpool.tile([128, NBLK, 96], F32, name='emn', tag='phA')
                nc.vector.tensor_scalar_min(emn[:], t96[:], 0.0)
                nc.scalar.activation(emn[:], emn[:], AF.Exp)
                nc.vector.tensor_scalar_add(emn[:], emn[:], -1.0)
                nc.vector.tensor_scalar_max(t96[:], t96[:], 0.0)
                nc.vector.tensor_tensor(out=t96[:], in0=t96[:], in1=emn[:], op=OP.add)
                nc.vector.tensor_tensor(out=t96[:], in0=t96[:], in1=h_cur[:], op=OP.add)
                # LN
                mean = spool.tile([128, NBLK], F32, name='mean', tag='mean')
                nc.vector.tensor_reduce(mean[:], t96[:], axis=mybir.AxisListType.X, op=OP.add)
                nc.vector.tensor_scalar_mul(mean[:], mean[:], 1.0 / 96)
                nc.vector.tensor_tensor(out=t96[:], in0=t96[:],
                                        in1=mean[:].broadcast_to([128, NBLK, 96]),
                                        op=OP.subtract)
                sq = bpool.tile([128, NBLK, 96], F32, name='sq', tag='phA')
                nc.vector.tensor_tensor(out=sq[:], in0=t96[:], in1=t96[:], op=OP.mult)
                var = spool.tile([128, NBLK], F32, name='var', tag='var')
                nc.vector.tensor_reduce(var[:], sq[:], axis=mybir.AxisListType.X, op=OP.add)
                sd = spool.tile([128, NBLK], F32, name='sd', tag='sd')
                nc.scalar.activation(sd[:], var[:], AF.Sqrt, bias=eps_t[:, 0:1], scale=1.0 / 96)
                rstd = spool.tile([128, NBLK], F32, name='rstd', tag='rstd')
                nc.vector.reciprocal(rstd[:], sd[:])
                h_new = bpool.tile([128, NBLK, 96], F32, name=f'h{l + 1}', tag='h', bufs=2)
                nc.vector.tensor_tensor(out=t96[:], in0=t96[:],
                                        in1=rstd[:].broadcast_to([128, NBLK, 96]), op=OP.mult)
                nc.vector.tensor_tensor(out=t96[:], in0=t96[:],
                                        in1=_bmid(lns_t[:, l, :], NBLK), op=OP.mult)
                nc.vector.tensor_tensor(out=h_new[:], in0=t96[:],
                                        in1=_bmid(lnb_t[:, l, :], NBLK), op=OP.add)
                h_cur = h_new

            # ---------- readout ----------
            for b in range(NBLK):
                nb = 128 if b < NBLK - 1 else LASTN
                tbf2 = spool.tile([128, 96], BF16, name='tbf2', tag='tbf', bufs=3)
                nc.scalar.activation(tbf2[0:nb, :], h_cur[0:nb, b, :], AF.Copy)
                nc.sync.dma_start(table_loc[b * 128:b * 128 + nb, 0:96], tbf2[0:nb, :])
            nc.gpsimd.collective_compute(
                'AllGather', OP.bypass, replica_groups=rg,
                ins=[table_loc[0:NL, :].opt()], outs=[table.ap().opt()])

            zg = []
            for hh in range(2):
                zi = spool.tile([128, NZC * 8], I16, name=f'zi{hh}', tag=f'zi{hh}')
                nc.sync.dma_start(zi[:], zidx_e[hh][:])
                zgt = spool.tile([128, NZC, 128], BF16, name=f'zg{hh}', tag=f'zg{hh}')
                nc.gpsimd.dma_gather(zgt[:], table[hh * HALF:(hh + 1) * HALF, :], zi[:],
                                     NZC * 128, NZC * 128, 128, queue_num=gq[0] % 4)
                gq[0] += 1
                zg.append(zgt)

            z_T = spool.tile([97, NZC * 128], F32, name='z_T', tag='z_T')
            nc.vector.memset(z_T[96:97, :], 1.0)
            for jc in range(NZC):
                pz = pp.tile([128, 96], F32, name='pz', tag='pA', bufs=1)
                keys = [k for k in perm_keys if k[2] == jc]
                for i, (hh, ic, _) in enumerate(keys):
                    pi = perm_keys.index((hh, ic, jc))
                    pm = spool.tile([128, 128], BF16, name='pm', tag='pm', bufs=2)
                    nc.sync.dma_start(pm[:], perms_e[pi * 128:(pi + 1) * 128, :])
                    nc.tensor.matmul(pz[:], pm[:], zg[hh][:, ic, 0:96],
                                     start=(i == 0), stop=(i == len(keys) - 1))
                zs = spool.tile([128, 96], F32, name='zs', tag='zs', bufs=2)
                nc.vector.tensor_copy(zs[:], pz[:])
                ptz = pp.tile([96, 128], F32, name='ptz', tag='pT', bufs=1)
                nc.tensor.transpose(ptz[:], zs[:], ident_t[:])
                nc.vector.tensor_copy(z_T[0:96, jc * 128:(jc + 1) * 128], ptz[:])

            outS = spool.tile([128, NZC, 4], F32, name='outS', tag='outS')
            ooff = 0
            for hi_ in range(3):
                o = head_dims[hi_]
                w1t = spool.tile([97, 96], F32, name='w1t', tag='w1t', bufs=2)
                nc.sync.dma_start(w1t[:], hw[hi_][0][:])
                w2t = spool.tile([97, 48], F32, name='w2t', tag='w2t', bufs=2)
                nc.sync.dma_start(w2t[:], hw[hi_][1][:])
                w3t = spool.tile([48, o], F32, name='w3t', tag='w3t', bufs=2)
                nc.sync.dma_start(w3t[:], hw[hi_][2][0:48, :])
                b3t = spool.tile([4, 1], F32, name='b3t', tag='b3t', bufs=2)
                nc.sync.dma_start(b3t[0:o, :], hw[hi_][2][48:49, 0:o].rearrange('a b -> b a'))
                p1 = pp.tile([96, NZC * 128], F32, name='p1', tag='pT', bufs=1)
                nc.tensor.matmul(p1[:], w1t[:], z_T[:], start=True, stop=True)
                t1 = spool.tile([97, NZC * 128], F32, name='t1', tag='t1', bufs=2)
                nc.scalar.activation(t1[0:96, :], p1[:], AF.Relu)
                nc.vector.memset(t1[96:97, :], 1.0)
                p2 = pp.tile([48, NZC * 128], F32, name='p2', tag='pA', bufs=1)
                nc.tensor.matmul(p2[:], w2t[:], t1[:], start=True, stop=True)
                t2 = spool.tile([48, NZC * 128], F32, name='t2', tag='t2', bufs=2)
                nc.scalar.activation(t2[:], p2[:], AF.Relu)
                p3 = pp.tile([4, NZC * 128], F32, name='p3', tag='pD', bufs=2)
                nc.tensor.matmul(p3[0:o, :], w3t[:], t2[:], start=True, stop=True)
                oh = spool.tile([4, NZC * 128], F32, name='oh', tag='oh', bufs=2)
                nc.vector.tensor_scalar(out=oh[0:o, :], in0=p3[0:o, :],
                                        scalar1=b3t[0:o, 0:1], scalar2=None, op0=OP.add)
                for jc in range(NZC):
                    po = pp.tile([128, 4], F32, name='po', tag='pB', bufs=2)
                    nc.tensor.transpose(po[:, 0:o], oh[0:o, jc * 128:(jc + 1) * 128],
                                        ident_t[0:o, 0:o])
                    nc.vector.tensor_copy(outS[:, jc, ooff:ooff + o], po[:, 0:o])
                ooff += o
            nc.sync.dma_start(
                out_e.ap().rearrange('a z f -> (a z) f')
                    .rearrange('(c p) f -> p c f', p=128), outS[:])

    nc.compile()
    return nc


def _run(inputs, trace=False):
    N = int(np.asarray(inputs['x']).shape[0])
    E = int(np.asarray(inputs['edge_index']).shape[1])
    in_maps, meta = host_prep(inputs, N, E)
    nc = build(meta)
    res = run_bass_kernel_spmd(nc, in_maps, core_ids=list(range(NCORES)), trace=trace)
    return np.asarray(res.results[0]['out'], np.float32).reshape(64, 6, 4), res


def kernel(**inputs):
    return _run(inputs, trace=False)[0]


# revision 7
# speedup vs baseline: 1.1103x; 1.0246x over previous
"""AirportGNN (4-layer GAT, N=50000, E=800000) on 8 TRN2 NeuronCores.

Sharding: nodes 6250/core; edges assigned to the dst-owner core, dst-sorted,
128-edge chunks aligned to 128-node blocks, split into 2 streams by src<N/2
(dma_gather int16 index limit). All cores run ONE graph: per-(block,stream)
chunk counts are padded to the cross-core max so the structure is uniform.

Per layer:
  phase A: [hp|a_s|a_d] = h @ [W | W@Asrc | W@Adst] per local node -> 512B-row
           node table -> AllGather; a_d block rows kept in SBUF (adw_sb).
  phase B: dma_gather src rows per chunk (4 SWDGE queues). One-hot dst
           indicators are GENERATED ON CHIP (not loaded): Ms[e,d] via
           is_equal(dstcol, iota) on vector; Mt[d,e] via a K=1 PE broadcast
           of the dst row into PSUM + is_equal against the partition iota.
           alpha = a_s[src] + a_d[dst] (Mt matmul) + a_e (bf16, layer-major
           contiguous); leaky-relu via exp/max; segment softmax-sum via
           matmul against Ms accumulated in per-block PSUM; evacuated by the
           scalar engine to per-stream staging.
  epilogue (batched over blocks): self-loop term added densely, normalize,
           +bias, ELU, +residual, LayerNorm.
Readout: final table AllGather; zone rows gathered per half + reordered via
one-hot perm matmuls; 3 MLP heads in feature-major layout; core 0's output.
"""
import os
import sys
import numpy as np

if '/opt/trn_rl_repo' not in sys.path:
    sys.path.insert(0, '/opt/trn_rl_repo')

import ml_dtypes
import concourse.bacc as bacc
import concourse.mybir as mybir
import concourse.tile as tile
from concourse.ap import AP
from concourse.bass_utils import run_bass_kernel_spmd
from concourse._compat import cdiv

F32 = mybir.dt.float32
BF16 = mybir.dt.bfloat16
I16 = mybir.dt.int16
AF = mybir.ActivationFunctionType
OP = mybir.AluOpType

NCORES = 8
HID, H, C = 96, 4, 24
NEG = 0.2
GRP = 16
LAYERS = 4


def _bmid(ap, n, axis=1):
    """Insert a 0-step broadcast dim of size n at position axis of an AP."""
    l = [list(x) for x in ap.ap]
    return AP(ap.tensor, ap.offset, l[:axis] + [[0, n]] + l[axis:])


def _btail(ap, n):
    """Replace a trailing size-1 dim with a 0-step broadcast dim of size n."""
    l = [list(x) for x in ap.ap]
    assert l[-1][1] == 1, l
    return AP(ap.tensor, ap.offset, l[:-1] + [[0, n]])


def _wrap_idx(ix, n):
    a = np.zeros((16, cdiv(n, 16)), np.int16)
    ix = np.asarray(ix, np.int64)
    for p in range(16):
        v = ix[p::16]
        a[p, :len(v)] = v.astype(np.int16)
    return np.tile(a, (8, 1))


def host_prep(inputs, N, E):
    NL = N // NCORES
    HALF = N // 2
    NBLK = cdiv(NL, 128)
    LASTN = NL - (NBLK - 1) * 128

    x = np.asarray(inputs['x'], np.float32)
    ei = np.asarray(inputs['edge_index'], np.int64)
    ea_np = np.asarray(inputs['edge_attr'], np.float32)
    zone = np.asarray(inputs['zone_idx'], np.int64)
    src_all, dst_all = ei[0], ei[1]
    f32 = lambda k: np.asarray(inputs[k], np.float32)

    # ---- per-core raw edge lists, grouped by (core, block, stream) ----
    raw = []  # raw[k][s][b] = (src, dst_rel, ea)
    for k in range(NCORES):
        lo = k * NL
        sel = (dst_all >= lo) & (dst_all < lo + NL)
        es, ed = src_all[sel], dst_all[sel] - lo
        o = np.argsort(ed, kind='stable')
        es, ed, eat = es[o], ed[o], ea_np[sel][o]
        per = [[None] * NBLK for _ in range(2)]
        for s in (0, 1):
            for b in range(NBLK):
                m = (ed // 128 == b) & ((es < HALF) == (s == 0))
                bs, bd, be = es[m], ed[m] - b * 128, eat[m]
                o2 = np.argsort(bs, kind='stable')
                per[s][b] = (bs[o2], bd[o2], be[o2])
        raw.append(per)

    NWIN = NBLK
    ncb = [[max(cdiv(max(len(raw[k][s][b][0]), 1), 128) for k in range(NCORES))
            for b in range(NBLK)] for s in range(2)]
    nch = []
    for s in (0, 1):
        tot = sum(ncb[s])
        pad = (-tot) % GRP
        ncb[s][NBLK - 1] += pad
        nch.append(tot + pad)
    ncht = nch[0] + nch[1]
    chunk_win = []
    win_ranges = [[], []]
    for s in (0, 1):
        c = 0
        for b in range(NBLK):
            win_ranges[s].append((c, c + ncb[s][b]))
            chunk_win += [b] * ncb[s][b]
            c += ncb[s][b]

    # ---- shared weights ----
    W1a = np.concatenate([f32('in_w1'), f32('in_b1')[None, :]], 0)
    W2a = np.concatenate([f32('in_w2'), f32('in_b2')[None, :]], 0)
    Wcat = np.zeros((HID, LAYERS * 104), np.float32)
    w_eh = np.zeros((4, LAYERS, H), np.float32)
    bias_b = np.zeros((128, LAYERS * 96), np.float32)
    lns_b = np.zeros((128, LAYERS * 96), np.float32)
    lnb_b = np.zeros((128, LAYERS * 96), np.float32)
    for l in range(LAYERS):
        W = f32('conv_w')[l]
        As = np.zeros((HID, H), np.float32)
        Ad = np.zeros((HID, H), np.float32)
        for hh in range(H):
            As[hh * C:(hh + 1) * C, hh] = f32('conv_att_src')[l, hh]
            Ad[hh * C:(hh + 1) * C, hh] = f32('conv_att_dst')[l, hh]
        Wcat[:, l * 104:l * 104 + 96] = W
        Wcat[:, l * 104 + 96:l * 104 + 100] = W @ As
        Wcat[:, l * 104 + 100:l * 104 + 104] = W @ Ad
        w_eh[:, l, :] = np.einsum('ahc,hc->ah',
                                  f32('conv_lin_edge')[l].reshape(4, H, C),
                                  f32('conv_att_edge')[l])
        bias_b[:, l * 96:(l + 1) * 96] = f32('conv_bias')[l][None, :]
        lns_b[:, l * 96:(l + 1) * 96] = f32('norm_scale')[l][None, :]
        lnb_b[:, l * 96:(l + 1) * 96] = f32('norm_bias')[l][None, :]
    weh_b = np.tile(w_eh.reshape(1, 4 * LAYERS * H), (128, 1)).astype(np.float32)

    def head_aug(pre):
        return (np.concatenate([f32(pre + '_w1'), f32(pre + '_b1')[None]], 0),
                np.concatenate([f32(pre + '_w2'), f32(pre + '_b2')[None]], 0),
                np.concatenate([f32(pre + '_w3'), f32(pre + '_b3')[None]], 0))
    heads = [head_aug('cong'), head_aug('delay'), head_aug('jit')]

    ident = np.eye(128, dtype=np.float32)
    iota_bf = np.tile(np.arange(128, dtype=ml_dtypes.bfloat16), (128, 1))
    iotap_bf = np.arange(128, dtype=ml_dtypes.bfloat16).reshape(128, 1)
    ones1 = np.ones((1, 128), ml_dtypes.bfloat16)

    # ---- zone prep (shared) ----
    nz = len(zone)
    NZC = cdiv(nz, 128)
    zlists = [[j for j in range(nz) if (zone[j] < HALF) == (h == 0)] for h in range(2)]
    zidx = []
    for h in range(2):
        ix = [int(zone[j]) - h * HALF for j in zlists[h]]
        ix += [0] * (NZC * 128 - len(ix))
        zidx.append(_wrap_idx(ix, NZC * 128))
    perms = {}
    for h in range(2):
        for r, j in enumerate(zlists[h]):
            key = (h, r // 128, j // 128)
            if key not in perms:
                perms[key] = np.zeros((128, 128), np.float32)
            perms[key][r % 128, j % 128] = 1.0
    perm_keys = sorted(perms.keys())
    perm_mats = (np.concatenate([perms[k] for k in perm_keys], 0)
                 if perm_keys else np.zeros((128, 128), np.float32))

    # ---- per-core arrays ----
    in_maps = []
    for k in range(NCORES):
        ea_chunk = np.zeros((128, ncht, 4), np.float32)
        flat_idx = np.zeros((ncht, 128), np.int64)
        dstc = np.full((128, ncht), -1.0, ml_dtypes.bfloat16)
        dstrow = np.full((1, ncht * 128), -1.0, ml_dtypes.bfloat16)
        for s in (0, 1):
            off = 0 if s == 0 else nch[0]
            for w in range(NWIN):
                bsrc, bdst, bea = raw[k][s][w]
                c0 = win_ranges[s][w][0] + off
                for i in range(0, len(bsrc), 128):
                    ci = c0 + i // 128
                    n = min(128, len(bsrc) - i)
                    flat_idx[ci, :n] = bsrc[i:i + n] - s * HALF
                    dstc[:n, ci] = bdst[i:i + n].astype(ml_dtypes.bfloat16)
                    dstrow[0, ci * 128:ci * 128 + n] = \
                        bdst[i:i + n].astype(ml_dtypes.bfloat16)
                    ea_chunk[:n, ci, :] = bea[i:i + n]
        comb = np.zeros((128, (ncht // GRP) * 144), np.int16)
        for g in range(ncht // GRP):
            for hcall in range(2):
                ix = flat_idx[g * GRP + hcall * 8:g * GRP + (hcall + 1) * 8].reshape(-1)
                comb[:, g * 144 + hcall * 64:g * 144 + (hcall + 1) * 64] = \
                    _wrap_idx(ix, 8 * 128)
            comb[:, g * 144 + 128:g * 144 + 144] = \
                dstc[:, g * GRP:(g + 1) * GRP].view(np.int16)

        deg = np.bincount(dst_all[(dst_all >= k * NL) & (dst_all < (k + 1) * NL)] - k * NL,
                          minlength=NL).astype(np.float32)
        inv_deg = (1.0 / np.clip(deg, 1.0, None)).astype(np.float32)
        inv_deg_b = np.pad(inv_deg, (0, NBLK * 128 - NL)).reshape(NBLK, 128).T.copy()

        xa = x[k * NL:(k + 1) * NL]
        xT_aug = np.ascontiguousarray(
            np.concatenate([xa.T, np.ones((1, NL), np.float32)], 0))

        im = {
            'xT_aug': xT_aug, 'comb': comb,
            'dstrow': np.ascontiguousarray(dstrow),
            'ea_chunk': np.ascontiguousarray(ea_chunk.reshape(128, ncht * 4)),
            'ea_bf': np.ascontiguousarray(
                ea_chunk.reshape(128, ncht * 4).astype(ml_dtypes.bfloat16)),
            'inv_deg': np.ascontiguousarray(inv_deg_b),
            'W1a': W1a, 'W2a': W2a, 'Wcat': Wcat, 'weh': weh_b,
            'bias_b': bias_b, 'lns_b': lns_b, 'lnb_b': lnb_b,
            'ident': ident, 'iota_bf': iota_bf, 'iotap_bf': iotap_bf,
            'ones1': ones1,
            'zidx0': zidx[0], 'zidx1': zidx[1],
            'perms': perm_mats.astype(ml_dtypes.bfloat16),
        }
        for hi_, (w1, w2, w3) in enumerate(heads):
            im[f'hw1_{hi_}'], im[f'hw2_{hi_}'], im[f'hw3_{hi_}'] = w1, w2, w3
        in_maps.append(im)

    meta = {
        'N': N, 'NL': NL, 'HALF': HALF, 'NBLK': NBLK, 'LASTN': LASTN, 'NZC': NZC,
        'NWIN': NWIN,
        'nch': nch, 'ncht': ncht, 'chunk_win': chunk_win, 'win_ranges': win_ranges,
        'perm_keys': perm_keys, 'nperm': max(len(perm_keys), 1),
        'head_dims': [2, 1, 1],
    }
    return in_maps, meta


def build(meta):
    N, NL, HALF = meta['N'], meta['NL'], meta['HALF']
    NBLK, LASTN, NZC = meta['NBLK'], meta['LASTN'], meta['NZC']
    nch, ncht = meta['nch'], meta['ncht']
    chunk_win, win_ranges = meta['chunk_win'], meta['win_ranges']
    NWIN = meta['NWIN']
    perm_keys, nperm = meta['perm_keys'], meta['nperm']
    head_dims = meta['head_dims']

    nc = bacc.Bacc('TRN2', target_bir_lowering=False, debug=False, num_swdge_queues=4)
    P = lambda n, s, d, o=False: nc.declare_dram_parameter(n, s, d, isOutput=o)

    xT_aug = P('xT_aug', [13, NL], F32)
    comb_e = P('comb', [128, (ncht // GRP) * 144], I16)
    dstrow_e = P('dstrow', [1, ncht * 128], BF16)
    ea_e = P('ea_chunk', [128, ncht * 4], F32)
    eabf_e = P('ea_bf', [128, ncht * 4], BF16)
    inv_deg_e = P('inv_deg', [128, NBLK], F32)
    W1a_e = P('W1a', [13, 96], F32)
    W2a_e = P('W2a', [97, 96], F32)
    Wcat_e = P('Wcat', [HID, LAYERS * 104], F32)
    weh_e = P('weh', [128, 64], F32)
    bias_e = P('bias_b', [128, LAYERS * 96], F32)
    lns_e = P('lns_b', [128, LAYERS * 96], F32)
    lnb_e = P('lnb_b', [128, LAYERS * 96], F32)
    ident_e = P('ident', [128, 128], F32)
    iota_e = P('iota_bf', [128, 128], BF16)
    iotap_e = P('iotap_bf', [128, 1], BF16)
    ones1_e = P('ones1', [1, 128], BF16)
    zidx_e = [P('zidx0', [128, NZC * 8], I16), P('zidx1', [128, NZC * 8], I16)]
    perms_e = P('perms', [nperm * 128, 128], BF16)
    hw = [(P(f'hw1_{i}', [97, 96], F32), P(f'hw2_{i}', [97, 48], F32),
           P(f'hw3_{i}', [49, head_dims[i]], F32)) for i in range(3)]
    out_e = P('out', [64, 6, 4], F32, o=True)

    table_loc = nc.dram_tensor('table_loc', [NWIN * 128, 128], BF16)
    table = nc.dram_tensor('table', [N, 128], BF16, addr_space='Shared')
    # a_e logits, bf16, layer-major: [128, L, ncht, 4]
    ae_dram = nc.dram_tensor('ae_dram', [128, LAYERS * ncht * 4], BF16)
    rg = [list(range(NCORES))]
    ae_view = ae_dram.ap().rearrange('p (l c a) -> p l c a', l=LAYERS, a=4)

    with tile.TileContext(nc) as tc:
        with tc.tile_pool(name='const', bufs=1) as cpool, \
             tc.tile_pool(name='big', bufs=1) as bpool, \
             tc.tile_pool(name='st', bufs=1) as spool, \
             tc.tile_pool(name='ps', bufs=1, space='PSUM') as pp:

            def ctile(name, src_ap, shape, dt=F32):
                t = cpool.tile(shape, dt, name=name, tag=name)
                nc.sync.dma_start(t[:], src_ap)
                return t

            ident_t = ctile('ident_t', ident_e[:], [128, 128])
            weh_t = ctile('weh_t', weh_e[:], [128, 64])
            inv_deg_t = ctile('inv_deg_t', inv_deg_e[:], [128, NBLK])
            iota_t = ctile('iota_t', iota_e[:], [128, 128], BF16)
            iotap_t = ctile('iotap_t', iotap_e[:], [128, 1], BF16)
            ones1_t = ctile('ones1_t', ones1_e[:], [1, 128], BF16)
            Wcat_t = ctile('Wcat_t', Wcat_e[:].rearrange('p (l o) -> p l o', l=LAYERS),
                           [HID, LAYERS, 104])
            bias_t = ctile('bias_t', bias_e[:].rearrange('p (l o) -> p l o', l=LAYERS),
                           [128, LAYERS, 96])
            lns_t = ctile('lns_t', lns_e[:].rearrange('p (l o) -> p l o', l=LAYERS),
                          [128, LAYERS, 96])
            lnb_t = ctile('lnb_t', lnb_e[:].rearrange('p (l o) -> p l o', l=LAYERS),
                          [128, LAYERS, 96])

            h_cur = bpool.tile([128, NBLK, 96], F32, name='h0', tag='h', bufs=2)
            gq = [0]  # global gather counter: queue = i%4 must track Tile's DMASW sem i%8
            eps_t = cpool.tile([128, 1], F32, name='eps_t', tag='eps_t')
            nc.vector.memset(eps_t[:], 1e-5)
            loop_attr = bpool.tile([128, NBLK, 4], F32, name='loop_attr')
            loop_alpha = bpool.tile([128, NBLK, 16], F32, name='loop_alpha')

            # ---------- init-only work (big tiles share stg slots, dead by phase B) ----------
            zt = spool.tile([128, 128], BF16, name='zt', tag='zt')
            nc.vector.memset(zt[:], 0.0)
            for b in range(NWIN):
                nc.sync.dma_start(table_loc[b * 128:(b + 1) * 128, :], zt[:])

            # ---------- input MLP ----------
            t_T = bpool.tile([97, NL], F32, name='t_T', tag='stg0')
            W1a_t = ctile('W1a_t', W1a_e[:], [13, 96])
            W2a_t = ctile('W2a_t', W2a_e[:], [97, 96])
            xT_t = bpool.tile([13, NL], F32, name='xT_t', tag='stg1')
            nc.sync.dma_start(xT_t[:], xT_aug[:])
            for i in range(cdiv(NL, 512)):
                w = min(512, NL - i * 512)
                ps1 = pp.tile([96, 512], F32, name='ps1', tag='pT', bufs=1)
                nc.tensor.matmul(ps1[:, 0:w], W1a_t[:], xT_t[:, i * 512:i * 512 + w],
                                 start=True, stop=True)
                nc.scalar.activation(t_T[0:96, i * 512:i * 512 + w], ps1[:, 0:w], AF.Relu)
            nc.vector.memset(t_T[96:97, :], 1.0)
            for b in range(NBLK):
                nb = 128 if b < NBLK - 1 else LASTN
                ps2 = pp.tile([128, 96], F32, name='ps2', tag='pA', bufs=1)
                nc.tensor.matmul(ps2[0:nb, :], t_T[:, b * 128:b * 128 + nb], W2a_t[:],
                                 start=True, stop=True)
                if nb < 128:
                    nc.vector.memset(h_cur[96:128, b, :], 0.0)
                nc.vector.tensor_copy(h_cur[0:nb, b, :], ps2[0:nb, :])

            # ---------- a_e precompute (bf16, layer-major) ----------
            for g in range(cdiv(ncht, 32)):
                c0, c1 = g * 32, min((g + 1) * 32, ncht)
                w = c1 - c0
                eat = spool.tile([128, 32, 4], F32, name='eat', tag='eat', bufs=2)
                nc.sync.dma_start(eat[:, 0:w, :],
                                  ea_e[:].rearrange('p (c a) -> p c a', a=4)[:, c0:c1, :])
                aet = spool.tile([128, 32, 16], F32, name='aet', tag='aet', bufs=2)
                tmp = spool.tile([128, 32, 16], F32, name='aetmp', tag='aetmp', bufs=2)
                for a in range(4):
                    dst = aet if a == 0 else tmp
                    nc.vector.tensor_tensor(
                        out=dst[:, 0:w, :],
                        in0=_btail(eat[:, 0:w, a:a + 1], 16),
                        in1=_bmid(weh_t[:, a * 16:(a + 1) * 16], w), op=OP.mult)
                    if a > 0:
                        nc.vector.tensor_tensor(out=aet[:, 0:w, :], in0=aet[:, 0:w, :],
                                                in1=tmp[:, 0:w, :], op=OP.add)
                for l in range(LAYERS):
                    abf = spool.tile([128, 32, 4], BF16, name='abf', tag='abf', bufs=3)
                    nc.scalar.activation(abf[:, 0:w, :], aet[:, 0:w, l * 4:(l + 1) * 4],
                                         AF.Copy)
                    nc.sync.dma_start(
                        ae_view[:, l, c0:c1, :].rearrange('p c a -> p (c a)'),
                        abf[:, 0:w, :])

            # ---------- layers ----------
            for l in range(LAYERS):
                wid = 104 if l == 0 else 100

                # ---- phase A ----
                phA = bpool.tile([128, NBLK, 104], F32, name=f'phA{l}', tag='phA')
                adw_sb = bpool.tile([128, NWIN, 4], BF16, name=f'adw{l}', tag='adw_sb',
                                    bufs=2)
                nc.vector.memset(adw_sb[:], 0.0)
                for b in range(NBLK):
                    nb = 128 if b < NBLK - 1 else LASTN
                    pt = pp.tile([96, 128], F32, name='pt', tag='pT', bufs=1)
                    nc.tensor.transpose(pt[:], h_cur[:, b, :], ident_t[:])
                    hT = spool.tile([96, 128], F32, name='hT', tag='hT', bufs=3)
                    nc.vector.tensor_copy(hT[:], pt[:])
                    pa = pp.tile([128, 104], F32, name='pa', tag='pA', bufs=1)
                    nc.tensor.matmul(pa[0:nb, :], hT[:, 0:nb], Wcat_t[:, l, :],
                                     start=True, stop=True)
                    if nb < 128:
                        nc.vector.memset(phA[96:128, b, :], 0.0)
                    nc.vector.tensor_copy(phA[0:nb, b, :], pa[0:nb, :])
                    nc.scalar.activation(adw_sb[0:nb, b, :], pa[0:nb, 100:104], AF.Copy)
                    tbf = spool.tile([128, 104], BF16, name='tbf', tag='tbf', bufs=3)
                    nc.scalar.activation(tbf[0:nb, :], pa[0:nb, :], AF.Copy)
                    nc.sync.dma_start(table_loc[b * 128:b * 128 + nb, 0:104],
                                      tbf[0:nb, :])
                nc.gpsimd.collective_compute(
                    'AllGather', OP.bypass, replica_groups=rg,
                    ins=[table_loc[0:NL, :].opt()], outs=[table.ap().opt()])

                # ---- phase B ----
                stg = [bpool.tile([128, NBLK, 104], F32, name=f'stg{s}_{l}', tag=f'stg{s}')
                       for s in (0, 1)]
                nc.vector.memset(stg[0][:], 0.0)
                nc.vector.memset(stg[1][:], 0.0)
                for s in (0, 1):
                    goff = (0 if s == 0 else nch[0] // GRP)
                    coff = 0 if s == 0 else nch[0]
                    tbl_half = table[s * HALF:(s + 1) * HALF, :]
                    cur_ps, cur_b = None, -1
                    for g in range(nch[s] // GRP):
                        cg0 = coff + g * GRP
                        comb_t = spool.tile([128, 144], I16, name='comb_t', tag='comb_t',
                                            bufs=4)
                        nc.sync.dma_start(
                            comb_t[:],
                            comb_e[:, (goff + g) * 144:(goff + g + 1) * 144])
                        drow = spool.tile([1, GRP * 128], BF16, name='drow', tag='drow',
                                          bufs=4)
                        nc.scalar.dma_start(drow[:],
                                            dstrow_e[0:1, cg0 * 128:(cg0 + GRP) * 128])
                        aet2 = spool.tile([128, GRP, 4], BF16, name='aet2', tag='aet2',
                                          bufs=4)
                        nc.scalar.dma_start(
                            aet2[:], ae_view[:, l, cg0:cg0 + GRP, :]
                            .rearrange('p c a -> p (c a)'))
                        gt = spool.tile([128, GRP, 128], BF16, name='gt', tag='gt', bufs=2)
                        for half_g in range(GRP // 8):
                            nc.gpsimd.dma_gather(
                                gt[:, half_g * 8:(half_g + 1) * 8, :], tbl_half,
                                comb_t[:, half_g * 64:(half_g + 1) * 64], 1024, 1024,
                                128, queue_num=gq[0] % 4)
                            gq[0] += 1

                        # on-chip one-hot generation
                        Ms_sb = spool.tile([128, GRP, 128], BF16, name='Ms_sb',
                                           tag='Ms_sb', bufs=2)
                        dstc_ap = comb_t[:, 128:144].bitcast(BF16)
                        nc.vector.tensor_tensor(
                            out=Ms_sb[:],
                            in0=_btail(dstc_ap.rearrange('p (c o) -> p c o', o=1), 128),
                            in1=_bmid(iota_t[:], GRP), op=OP.is_equal)
                        Mt_sb = spool.tile([128, GRP, 128], BF16, name='Mt_sb',
                                           tag='Mt_sb', bufs=2)
                        for q in range(GRP // 4):
                            Dq = pp.tile([128, 512], F32, name='Dq', tag='pDq', bufs=2)
                            nc.tensor.matmul(Dq[:], ones1_t[0:1, :],
                                             drow[0:1, q * 512:(q + 1) * 512],
                                             start=True, stop=True)
                            nc.vector.tensor_tensor(
                                out=Mt_sb[:, q * 4:(q + 1) * 4, :],
                                in0=Dq[:].rearrange('p (c e) -> p c e', e=128),
                                in1=_bmid(_btail(iotap_t[:], 128), 4), op=OP.is_equal)

                        adp = pp.tile([128, GRP * 4], F32, name='adp', tag='pD', bufs=2)
                        for c in range(GRP):
                            w = chunk_win[cg0 + c]
                            nc.tensor.matmul(adp[:, c * 4:(c + 1) * 4], Mt_sb[:, c, :],
                                             adw_sb[:, w, :], start=True, stop=True)
                        alpha = spool.tile([128, GRP, 4], F32, name='alpha', tag='alpha', bufs=4)
                        nc.vector.tensor_tensor(
                            out=alpha[:], in0=gt[:, :, 96:100],
                            in1=adp[:].rearrange('p (c f) -> p c f', c=GRP), op=OP.add)
                        nc.vector.tensor_tensor(out=alpha[:], in0=alpha[:], in1=aet2[:],
                                                op=OP.add)
                        msg = spool.tile([128, GRP, 104], BF16, name='msg', tag='msg', bufs=2)
                        e1 = spool.tile([128, GRP, 4], F32, name='e1', tag='e1', bufs=4)
                        nc.scalar.activation(e1[:], alpha[:], AF.Exp)
                        e2 = spool.tile([128, GRP, 4], F32, name='e2', tag='e2', bufs=4)
                        nc.scalar.activation(e2[:], alpha[:], AF.Exp, scale=NEG)
                        ex = spool.tile([128, GRP, 4], F32, name='ex', tag='ex', bufs=4)
                        nc.vector.tensor_tensor(out=ex[:], in0=e1[:], in1=e2[:], op=OP.max)
                        nc.scalar.activation(msg[:, :, 96:100], ex[:], AF.Copy)
                        nc.vector.tensor_tensor(
                            out=msg[:, :, 0:96].rearrange('p c (h r) -> p c h r', h=4),
                            in0=gt[:, :, 0:96].rearrange('p c (h r) -> p c h r', h=4),
                            in1=ex[:].broadcast_to([128, GRP, 4, 24]), op=OP.mult)
                        if l == 0:
                            nc.sync.dma_start(
                                msg[:, :, 100:104],
                                eabf_e[:].rearrange('p (c a) -> p c a', a=4)
                                [:, cg0:cg0 + GRP, :])
                        for c in range(GRP):
                            cb = chunk_win[cg0 + c]
                            if cb != cur_b:
                                assert cur_ps is None
                                cur_ps = pp.tile([128, 104], F32, name='psb', tag='pB', bufs=2)
                                cur_b = cb
                            first = (cg0 + c) == coff + win_ranges[s][cb][0]
                            last = (cg0 + c) == coff + win_ranges[s][cb][1] - 1
                            nc.tensor.matmul(cur_ps[:, 0:wid], Ms_sb[:, c, :], msg[:, c, 0:wid],
                                             start=first, stop=last)
                            if last:
                                nc.scalar.activation(stg[s][:, cb, 0:wid], cur_ps[:, 0:wid],
                                                     AF.Copy)
                                cur_ps, cur_b = None, -1
                    assert cur_ps is None

                # ---- epilogue (batched over blocks) ----
                nc.vector.tensor_tensor(out=stg[0][:], in0=stg[0][:], in1=stg[1][:], op=OP.add)
                sA = stg[0]
                if l == 0:
                    nc.vector.tensor_tensor(
                        out=loop_attr[:], in0=sA[:, :, 100:104],
                        in1=inv_deg_t[:].broadcast_to([128, NBLK, 4]), op=OP.mult)
                # loop_alpha[p,b,l*4+h] = sum_a loop_attr[p,b,a] * weh[a, l*4+h]
                la_tmp = spool.tile([128, NBLK, 16], F32, name='la_tmp', tag='la_tmp')
                for a in range(4):
                    dst = loop_alpha if a == 0 else la_tmp
                    nc.vector.tensor_tensor(
                        out=dst[:],
                        in0=_btail(loop_attr[:, :, a:a + 1], 16),
                        in1=_bmid(weh_t[:, a * 16:(a + 1) * 16], NBLK), op=OP.mult)
                    if a > 0:
                        nc.vector.tensor_tensor(out=loop_alpha[:], in0=loop_alpha[:],
                                                in1=la_tmp[:], op=OP.add)
                all_ = spool.tile([128, NBLK, 4], F32, name='all_', tag='all_')
                nc.vector.tensor_tensor(out=all_[:], in0=loop_alpha[:, :, l * 4:(l + 1) * 4],
                                        in1=phA[:, :, 96:100], op=OP.add)
                nc.vector.tensor_tensor(out=all_[:], in0=all_[:], in1=phA[:, :, 100:104],
                                        op=OP.add)
                lk2 = spool.tile([128, NBLK, 4], F32, name='lk2', tag='lk2')
                nc.vector.tensor_scalar_mul(lk2[:], all_[:], NEG)
                nc.vector.tensor_tensor(out=all_[:], in0=all_[:], in1=lk2[:], op=OP.max)
                exl = spool.tile([128, NBLK, 4], F32, name='exl', tag='exl')
                nc.scalar.activation(exl[:], all_[:], AF.Exp)
                # num += exl * hp ; den += exl
                t96 = bpool.tile([128, NBLK, 96], F32, name='t96', tag='stg1')
                nc.vector.tensor_tensor(
                    out=t96[:].rearrange('p b (h r) -> p b h r', h=4),
                    in0=phA[:, :, 0:96].rearrange('p b (h r) -> p b h r', h=4),
                    in1=exl[:].broadcast_to([128, NBLK, 4, 24]), op=OP.mult)
                nc.vector.tensor_tensor(out=sA[:, :, 0:96], in0=sA[:, :, 0:96], in1=t96[:],
                                        op=OP.add)
                nc.vector.tensor_tensor(out=sA[:, :, 96:100], in0=sA[:, :, 96:100],
                                        in1=exl[:], op=OP.add)
                rec = spool.tile([128, NBLK, 4], F32, name='rec', tag='rec')
                nc.vector.reciprocal(rec[:], sA[:, :, 96:100])
                # out = num*rec + bias
                nc.vector.tensor_tensor(
                    out=t96[:].rearrange('p b (h r) -> p b h r', h=4),
                    in0=sA[:, :, 0:96].rearrange('p b (h r) -> p b h r', h=4),
                    in1=rec[:].broadcast_to([128, NBLK, 4, 24]), op=OP.mult)
                nc.vector.tensor_tensor(out=t96[:], in0=t96[:],
                                        in1=_bmid(bias_t[:, l, :], NBLK), op=OP.add)
                # elu: (exp(min(x,0)) - 1) + max(x,0), then + res
                emn = bpool.tile([128, NBLK, 96], F32, name='emn', tag='phA')
                nc.vector.tensor_scalar_min(emn[:], t96[:], 0.0)
                nc.scalar.activation(emn[:], emn[:], AF.Exp)
                nc.vector.tensor_scalar_add(emn[:], emn[:], -1.0)
                nc.vector.tensor_scalar_max(t96[:], t96[:], 0.0)
                nc.vector.tensor_tensor(out=t96[:], in0=t96[:], in1=emn[:], op=OP.add)
                nc.vector.tensor_tensor(out=t96[:], in0=t96[:], in1=h_cur[:], op=OP.add)
                # LN
                mean = spool.tile([128, NBLK], F32, name='mean', tag='mean')
                nc.vector.tensor_reduce(mean[:], t96[:], axis=mybir.AxisListType.X, op=OP.add)
                nc.vector.tensor_scalar_mul(mean[:], mean[:], 1.0 / 96)
                nc.vector.tensor_tensor(out=t96[:], in0=t96[:],
                                        in1=mean[:].broadcast_to([128, NBLK, 96]),
                                        op=OP.subtract)
                sq = bpool.tile([128, NBLK, 96], F32, name='sq', tag='phA')
                nc.vector.tensor_tensor(out=sq[:], in0=t96[:], in1=t96[:], op=OP.mult)
                var = spool.tile([128, NBLK], F32, name='var', tag='var')
                nc.vector.tensor_reduce(var[:], sq[:], axis=mybir.AxisListType.X, op=OP.add)
                sd = spool.tile([128, NBLK], F32, name='sd', tag='sd')
                nc.scalar.activation(sd[:], var[:], AF.Sqrt, bias=eps_t[:, 0:1], scale=1.0 / 96)
                rstd = spool.tile([128, NBLK], F32, name='rstd', tag='rstd')
                nc.vector.reciprocal(rstd[:], sd[:])
                h_new = bpool.tile([128, NBLK, 96], F32, name=f'h{l + 1}', tag='h', bufs=2)
                nc.vector.tensor_tensor(out=t96[:], in0=t96[:],
                                        in1=rstd[:].broadcast_to([128, NBLK, 96]), op=OP.mult)
                nc.vector.tensor_tensor(out=t96[:], in0=t96[:],
                                        in1=_bmid(lns_t[:, l, :], NBLK), op=OP.mult)
                nc.vector.tensor_tensor(out=h_new[:], in0=t96[:],
                                        in1=_bmid(lnb_t[:, l, :], NBLK), op=OP.add)
                h_cur = h_new

            # ---------- readout ----------
            for b in range(NBLK):
                nb = 128 if b < NBLK - 1 else LASTN
                tbf2 = spool.tile([128, 96], BF16, name='tbf2', tag='tbf', bufs=3)
                nc.scalar.activation(tbf2[0:nb, :], h_cur[0:nb, b, :], AF.Copy)
                nc.sync.dma_start(table_loc[b * 128:b * 128 + nb, 0:96], tbf2[0:nb, :])
            nc.gpsimd.collective_compute(
                'AllGather', OP.bypass, replica_groups=rg,
                ins=[table_loc[0:NL, :].opt()], outs=[table.ap().opt()])

            zg = []
            for hh in range(2):
                zi = spool.tile([128, NZC * 8], I16, name=f'zi{hh}', tag=f'zi{hh}')
                nc.sync.dma_start(zi[:], zidx_e[hh][:])
                zgt = spool.tile([128, NZC, 128], BF16, name=f'zg{hh}', tag=f'zg{hh}')
                nc.gpsimd.dma_gather(zgt[:], table[hh * HALF:(hh + 1) * HALF, :], zi[:],
                                     NZC * 128, NZC * 128, 128, queue_num=gq[0] % 4)
                gq[0] += 1
                zg.append(zgt)

            z_T = spool.tile([97, NZC * 128], F32, name='z_T', tag='z_T')
            nc.vector.memset(z_T[96:97, :], 1.0)
            for jc in range(NZC):
                pz = pp.tile([128, 96], F32, name='pz', tag='pA', bufs=1)
                keys = [k for k in perm_keys if k[2] == jc]
                for i, (hh, ic, _) in enumerate(keys):
                    pi = perm_keys.index((hh, ic, jc))
                    pm = spool.tile([128, 128], BF16, name='pm', tag='pm', bufs=2)
                    nc.sync.dma_start(pm[:], perms_e[pi * 128:(pi + 1) * 128, :])
                    nc.tensor.matmul(pz[:], pm[:], zg[hh][:, ic, 0:96],
                                     start=(i == 0), stop=(i == len(keys) - 1))
                zs = spool.tile([128, 96], F32, name='zs', tag='zs', bufs=2)
                nc.vector.tensor_copy(zs[:], pz[:])
                ptz = pp.tile([96, 128], F32, name='ptz', tag='pT', bufs=1)
                nc.tensor.transpose(ptz[:], zs[:], ident_t[:])
                nc.vector.tensor_copy(z_T[0:96, jc * 128:(jc + 1) * 128], ptz[:])

            outS = spool.tile([128, NZC, 4], F32, name='outS', tag='outS')
            ooff = 0
            for hi_ in range(3):
                o = head_dims[hi_]
                w1t = spool.tile([97, 96], F32, name='w1t', tag='w1t', bufs=2)
                nc.sync.dma_start(w1t[:], hw[hi_][0][:])
                w2t = spool.tile([97, 48], F32, name='w2t', tag='w2t', bufs=2)
                nc.sync.dma_start(w2t[:], hw[hi_][1][:])
                w3t = spool.tile([48, o], F32, name='w3t', tag='w3t', bufs=2)
                nc.sync.dma_start(w3t[:], hw[hi_][2][0:48, :])
                b3t = spool.tile([4, 1], F32, name='b3t', tag='b3t', bufs=2)
                nc.sync.dma_start(b3t[0:o, :], hw[hi_][2][48:49, 0:o].rearrange('a b -> b a'))
                p1 = pp.tile([96, NZC * 128], F32, name='p1', tag='pT', bufs=1)
                nc.tensor.matmul(p1[:], w1t[:], z_T[:], start=True, stop=True)
                t1 = spool.tile([97, NZC * 128], F32, name='t1', tag='t1', bufs=2)
                nc.scalar.activation(t1[0:96, :], p1[:], AF.Relu)
                nc.vector.memset(t1[96:97, :], 1.0)
                p2 = pp.tile([48, NZC * 128], F32, name='p2', tag='pA', bufs=1)
                nc.tensor.matmul(p2[:], w2t[:], t1[:], start=True, stop=True)
                t2 = spool.tile([48, NZC * 128], F32, name='t2', tag='t2', bufs=2)
                nc.scalar.activation(t2[:], p2[:], AF.Relu)
                p3 = pp.tile([4, NZC * 128], F32, name='p3', tag='pD', bufs=2)
                nc.tensor.matmul(p3[0:o, :], w3t[:], t2[:], start=True, stop=True)
                oh = spool.tile([4, NZC * 128], F32, name='oh', tag='oh', bufs=2)
                nc.vector.tensor_scalar(out=oh[0:o, :], in0=p3[0:o, :],
                                        scalar1=b3t[0:o, 0:1], scalar2=None, op0=OP.add)
                for jc in range(NZC):
                    po = pp.tile([128, 4], F32, name='po', tag='pB', bufs=2)
                    nc.tensor.transpose(po[:, 0:o], oh[0:o, jc * 128:(jc + 1) * 128],
                                        ident_t[0:o, 0:o])
                    nc.vector.tensor_copy(outS[:, jc, ooff:ooff + o], po[:, 0:o])
                ooff += o
            nc.sync.dma_start(
                out_e.ap().rearrange('a z f -> (a z) f')
                    .rearrange('(c p) f -> p c f', p=128), outS[:])

    nc.compile()
    return nc


def _run(inputs, trace=False):
    N = int(np.asarray(inputs['x']).shape[0])
    E = int(np.asarray(inputs['edge_index']).shape[1])
    in_maps, meta = host_prep(inputs, N, E)
    nc = build(meta)
    res = run_bass_kernel_spmd(nc, in_maps, core_ids=list(range(NCORES)), trace=trace)
    return np.asarray(res.results[0]['out'], np.float32).reshape(64, 6, 4), res


def kernel(**inputs):
    return _run(inputs, trace=False)[0]
